# revision 3
# baseline (speedup 1.0000x reference)
# GCN (2-layer GCNConv + BatchNorm + ReLU + global mean pool) on 8 TRN2 cores.
#
# v4 strategy (dst-partitioned, no ReduceScatter):
#   * Edges partitioned by DST owner.  Every core holds the FULL raw-x row
#     table (host-replicated, pure permutation/padding) in DRAM, split in two
#     25088-row halves so int16 gather indices stay in range.
#   * Core k owns nodes [k*6250,(k+1)*6250); node l -> slot s (s = l, shifted
#     +459 for the upper shard half so each half owns 459 dedicated pad
#     slots); slot s -> partition s%128, group s//128 (56 groups, 28 pairs).
#   * Per-dst-occurrence rounds keep scatter rows unique per instruction.
#     Pipeline per round: Pool gather (f32, src-half substreams joined in one
#     buffer) -> DVE norm-mult (msg * norm_e broadcast over channels, bf16
#     out; norm = dinv[src]*dinv[dst] from index data, 0 on pads) -> Pool
#     scatter-add into an SBUF even/odd-group accumulator pair.
#   * Rounds big enough are split into shard halves A (groups 0..27) and
#     B (28..55); all A pieces run before B so A's aggs assembly + BN-stats
#     matmuls overlap B's edge work.  Mid rounds run unsplit (early).  Deep
#     rounds (small) scatter-add onto a zeroed DRAM strip (per-index cost, no
#     SBUF footprint floor) and fold in via one reload+add before assembly.
#   * aggs = acc + (dinv^2 * x_own + strip);  BN stats via the accumulated
#     A^T[A|1] matmul; cross-core stats via bf16 AllGather + on-chip tree
#     sum; one more bf16 AllGather for the pooled output partials.
#   * Layer 2 never materializes per-node features: out = (q . h1) W2 + b2
#     with q[u,g] built on the host from INDEX data only.
#
# Host-side preprocessing uses only index data (edge_index, batch): degrees,
# edge partitioning/rounds, per-edge norms, the q pooling matrix.  Feature
# data (x) is only permuted/padded on the host, never computed with.

import numpy as np

N_NODES = 50000
N_EDGES = 800000
D = 64
NCORES = 8
NUM_GRAPHS = 64
BN_EPS = 1e-5
SL = N_NODES // NCORES          # 6250 owned nodes per core
NT = 56                         # groups
SLP = 128 * NT                  # 7168 padded slots
HALF_G = NT // 2                # 28 groups per shard half
HALF_S = SLP // 2               # 3584 slots per half
HREAL = SL // 2                 # 3125 real nodes per half
HPAD = HALF_S - HREAL           # 459 pad slots per half
TROWS = 50176                   # padded table rows (2 * 25088)
THALF = TROWS // 2              # 25088 (int16-safe)
STRIP_G = NT + 3                # strip groups (56 real + 3 pad)
STRIP_ROWS = 128 * STRIP_G      # 7552
SBUF_MIN = 1500                 # min real rows for the SBUF scatter path


class Cfg:
    def __init__(self):
        # pieces: execution-ordered dicts
        #  {'kind':'S'|'D', 'n', 'nA', 'nB', 'cols':(c0,c1) pair-col slice,
        #   'stage':0 cross/1 halfA/2 halfB/-1 deep, 'goff','soff','noff'}
        self.pieces = []


LAST_EXEC_TIME_NS = None
_NC_CACHE = {}
_LAST_IN_MAPS = None


def build(cfg):
    import concourse.mybir as mybir
    import concourse.mybir as mb
    import concourse.tile as tile
    from concourse import bacc
    from concourse.masks import make_identity

    f32 = mybir.dt.float32
    bf16 = mybir.dt.bfloat16
    i16 = mybir.dt.int16
    NN = float(N_NODES)
    RG = [list(range(NCORES))]
    pieces = cfg.pieces
    tot_s = sum(p["n"] for p in pieces)
    SMAXC = max(p["n"] for p in pieces)

    nc = bacc.Bacc(
        "TRN2", target_bir_lowering=False, debug=False, num_devices=NCORES
    )

    # --- external inputs ---
    xr = nc.declare_dram_parameter("xr", [TROWS, D], f32, isOutput=False)
    xo = nc.declare_dram_parameter("xo", [128, NT * D], f32, isOutput=False)
    d2o = nc.declare_dram_parameter("d2o", [128, NT], f32, isOutput=False)
    qsl = nc.declare_dram_parameter("qsl", [128, NT * NUM_GRAPHS], bf16,
                                    isOutput=False)
    glo_d = nc.declare_dram_parameter("glo", [128, tot_s // 16], i16,
                                      isOutput=False)
    sct_d = nc.declare_dram_parameter("sct", [128, tot_s // 16], i16,
                                      isOutput=False)
    nrm_d = nc.declare_dram_parameter("nrm", [128, tot_s // 128], f32,
                                      isOutput=False)
    p1_d = nc.declare_dram_parameter("p1", [1, NUM_GRAPHS], f32, isOutput=False)
    w1_d = nc.declare_dram_parameter("w1", [D, D], f32, isOutput=False)
    b1_d = nc.declare_dram_parameter("b1", [D, 1], f32, isOutput=False)
    ga_d = nc.declare_dram_parameter("ga", [D, 1], f32, isOutput=False)
    be_d = nc.declare_dram_parameter("be", [D, 1], f32, isOutput=False)
    w2_d = nc.declare_dram_parameter("w2", [D, D], f32, isOutput=False)
    b2_d = nc.declare_dram_parameter("b2", [1, D], f32, isOutput=False)
    out_d = nc.declare_dram_parameter("out", [NUM_GRAPHS, D], f32,
                                      isOutput=True)

    # --- internal DRAM ---
    strip = nc.dram_tensor("strip", [STRIP_ROWS, 2 * D], bf16)
    sag_in = nc.dram_tensor("sag_in", [D, D + 1], bf16)
    sag_out = nc.dram_tensor("sag_out", [NCORES * D, D + 1], bf16,
                             addr_space="Shared")
    oag_in = nc.dram_tensor("oag_in", [NUM_GRAPHS, D], bf16)
    oag_out = nc.dram_tensor("oag_out", [NCORES * NUM_GRAPHS, D], bf16,
                             addr_space="Shared")

    xr_rows = xr[:, :]
    strip_rows = strip[:, :]

    with tile.TileContext(nc) as tc:
        with (
            tc.tile_pool(name="const", bufs=1) as const,
            tc.tile_pool(name="persist", bufs=1) as persist,
            tc.tile_pool(name="work", bufs=3) as work,
            tc.tile_pool(name="idxp", bufs=3) as idxp,
            tc.tile_pool(name="msgp", bufs=3) as msgp,
            tc.tile_pool(name="msgbp", bufs=3) as msgbp,
            tc.tile_pool(name="spsum", bufs=1, space="PSUM") as spsum,
            tc.tile_pool(name="wpsum", bufs=3, space="PSUM") as wpsum,
        ):
            # --- accumulators + zeroed strip (first: unblock stage-0/deep) ---
            own = persist.tile([128, HALF_G, D], bf16, name="own")    # even g
            peer = persist.tile([128, HALF_G, D], bf16, name="peer")  # odd g
            nc.vector.memset(own[:], 0.0)
            nc.vector.memset(peer[:], 0.0)
            zs = persist.tile([128, 8, 2 * D], bf16, name="zs")
            nc.vector.memset(zs[:], 0.0)
            epsc = const.tile([D, 1], f32)
            nc.vector.memset(epsc[:], BN_EPS)
            strip_v = strip_rows.rearrange("(g p) c -> p g c", p=128)
            for g0 in range(0, STRIP_G, 8):
                gn = min(8, STRIP_G - g0)
                nc.scalar.dma_start(out=strip_v[:, g0 : g0 + gn, :],
                                    in_=zs[:, :gn, :])
            # preload the Sqrt/Relu act tables off the critical path
            warm = const.tile([1, 1], f32)
            nc.scalar.activation(warm[:], epsc[0:1, :],
                                 mb.ActivationFunctionType.Sqrt)
            nc.scalar.activation(warm[:], epsc[0:1, :],
                                 mb.ActivationFunctionType.Relu)

            # --- batched idx/norm loads (chunked, execution order); chunk 0
            # on SP covers the whole stage-0 + deep prefix ---
            glo_s = persist.tile([128, tot_s // 16], i16, name="glo_s")
            sct_s = persist.tile([128, tot_s // 16], i16, name="sct_s")
            nrm_s = persist.tile([128, tot_s // 128], f32, name="nrm_s")
            ICHUNK = 1792
            bounds = [0, 256] + list(range(ICHUNK, tot_s // 16, ICHUNK)) + [
                tot_s // 16]
            for qi in range(len(bounds) - 1):
                c0, c1 = bounds[qi], bounds[qi + 1]
                if c0 >= c1:
                    continue
                eng = nc.sync if qi % 2 == 0 else nc.scalar
                eng.dma_start(out=glo_s[:, c0:c1], in_=glo_d[:, c0:c1])
                eng.dma_start(out=sct_s[:, c0:c1], in_=sct_d[:, c0:c1])
                d0, d1 = c0 // 8, min(c1 // 8, tot_s // 128)
                if qi == len(bounds) - 2:
                    d1 = tot_s // 128
                if d0 < d1:
                    eng.dma_start(out=nrm_s[:, d0:d1], in_=nrm_d[:, d0:d1])

            # --- constants (scalar queue, after the idx chunks) ---
            w1s = const.tile([D, D], f32)
            nc.scalar.dma_start(out=w1s[:], in_=w1_d[:, :])
            w2s = const.tile([D, D], f32)
            nc.scalar.dma_start(out=w2s[:], in_=w2_d[:, :])
            b1c = const.tile([D, 1], f32)
            nc.scalar.dma_start(out=b1c[:], in_=b1_d[:, :])
            gac = const.tile([D, 1], f32)
            nc.scalar.dma_start(out=gac[:], in_=ga_d[:, :])
            bec = const.tile([D, 1], f32)
            nc.scalar.dma_start(out=bec[:], in_=be_d[:, :])
            b2r = const.tile([1, D], f32)
            nc.scalar.dma_start(out=b2r[:], in_=b2_d[:, :])
            p1s = const.tile([1, NUM_GRAPHS], f32)
            nc.scalar.dma_start(out=p1s[:], in_=p1_d[:, :])
            d2s = const.tile([128, NT], f32)
            nc.scalar.dma_start(out=d2s[:], in_=d2o[:, :])
            xos = persist.tile([128, NT, D], f32, name="xos")
            nc.scalar.dma_start(
                out=xos[:], in_=xo[:, :].rearrange("p (g d) -> p g d", d=D)
            )
            qs = persist.tile([128, NT, NUM_GRAPHS], bf16, name="qs")
            nc.scalar.dma_start(
                out=qs[:], in_=qsl[:, :].rearrange("p (g d) -> p g d",
                                                   d=NUM_GRAPHS)
            )

            # --- edge pipeline ---
            def edge_piece(pc):
                n = pc["n"]
                nA = pc["nA"]
                so = pc["soff"]
                no = pc["noff"]
                msg = msgp.tile([128, SMAXC // 128, D], f32, tag="msg",
                                name="msg")
                for half, go, nseg in ((0, 0, nA), (1, nA, n - nA)):
                    if nseg == 0:
                        continue
                    base = half * THALF
                    nc.gpsimd.dma_gather(
                        out_ap=msg[:, go // 128 : (go + nseg) // 128, :],
                        in_ap=xr_rows[base : base + THALF, :],
                        idxs_ap=glo_s[:, so + go // 16 : so + (go + nseg) // 16],
                        num_idxs=nseg, num_idxs_reg=nseg, elem_size=D,
                        single_packet=False, queue_num=0,
                    )
                msgb = msgbp.tile([128, SMAXC // 128, D], bf16, tag="msgb",
                                  name="msgb")
                nc.vector.tensor_tensor(
                    out=msgb[:, : n // 128, :], in0=msg[:, : n // 128, :],
                    in1=nrm_s[:, no : no + n // 128].rearrange(
                        "p (g o) -> p g o", o=1).to_broadcast(
                        [128, n // 128, D]),
                    op=mybir.AluOpType.mult,
                )
                if pc["kind"] == "S":
                    c0, c1 = pc["cols"]
                    nc.gpsimd.dma_scatter_add(
                        own[:, c0:c1, :], msgb[:, : n // 128, :],
                        sct_s[:, so : so + n // 16], n, n, D,
                        sbuf_tokens_per_rank=128, parity_reg=0,
                        out_ap_other=peer[:, c0:c1, :],
                        single_packet=False, queue_num=0,
                    )
                else:
                    nc.gpsimd.dma_scatter_add(
                        strip_rows[:, 0:D], msgb[:, : n // 128, :],
                        sct_s[:, so : so + n // 16], n, n, D,
                        elem_step=2 * D,
                        single_packet=False, queue_num=0,
                    )

            def deep_bundle(pcs):
                """One gather stream + one norm-mult feeding several per-round
                strip scatters (avoids tiny chained pieces)."""
                n = sum(p["n"] for p in pcs)
                so0 = pcs[0]["soff"]
                no0 = pcs[0]["noff"]
                msg = msgp.tile([128, SMAXC // 128, D], f32, tag="msg",
                                name="msg")
                off = 0
                for pc in pcs:
                    for half, go, nseg in ((0, 0, pc["nA"]),
                                           (1, pc["nA"], pc["n"] - pc["nA"])):
                        if nseg == 0:
                            continue
                        base = half * THALF
                        o = off + go
                        nc.gpsimd.dma_gather(
                            out_ap=msg[:, o // 128 : (o + nseg) // 128, :],
                            in_ap=xr_rows[base : base + THALF, :],
                            idxs_ap=glo_s[:, so0 + o // 16 :
                                          so0 + (o + nseg) // 16],
                            num_idxs=nseg, num_idxs_reg=nseg, elem_size=D,
                            single_packet=False, queue_num=0,
                        )
                    off += pc["n"]
                msgb = msgbp.tile([128, SMAXC // 128, D], bf16, tag="msgb",
                                  name="msgb")
                nc.vector.tensor_tensor(
                    out=msgb[:, : n // 128, :], in0=msg[:, : n // 128, :],
                    in1=nrm_s[:, no0 : no0 + n // 128].rearrange(
                        "p (g o) -> p g o", o=1).to_broadcast(
                        [128, n // 128, D]),
                    op=mybir.AluOpType.mult,
                )
                off = 0
                for pc in pcs:
                    pn = pc["n"]
                    nc.gpsimd.dma_scatter_add(
                        strip_rows[:, 0:D],
                        msgb[:, off // 128 : (off + pn) // 128, :],
                        sct_s[:, so0 + off // 16 : so0 + (off + pn) // 16],
                        pn, pn, D, elem_step=2 * D,
                        single_packet=False, queue_num=0,
                    )
                    off += pn

            aggs = persist.tile([128, NT, D + 1], bf16, name="aggs")
            stats_ps = spsum.tile([D, D + 1], f32, name="stats_ps")
            tsl = persist.tile([128, NT, D], f32, name="tsl")
            agv = aggs[:, :, :D].rearrange("p (a two) d -> p a two d", two=2)
            tsv = tsl[:].rearrange("p (a two) d -> p a two d", two=2)

            def assemble(h):
                # chunked adds + stats matmuls so PE overlaps the DVE adds
                q = HALF_G // 2  # 14 pair-cols per half
                for ci in range(0, q, 4):
                    a0 = h * q + ci
                    a1 = min(a0 + 4, (h + 1) * q)
                    nc.vector.tensor_tensor(
                        out=agv[:, a0:a1, 0, :], in0=own[:, a0:a1, :],
                        in1=tsv[:, a0:a1, 0, :], op=mybir.AluOpType.add,
                    )
                    nc.vector.tensor_tensor(
                        out=agv[:, a0:a1, 1, :], in0=peer[:, a0:a1, :],
                        in1=tsv[:, a0:a1, 1, :], op=mybir.AluOpType.add,
                    )
                    for g in range(2 * a0, 2 * a1):
                        nc.tensor.matmul(
                            out=stats_ps[:], lhsT=aggs[:, g, :D],
                            rhs=aggs[:, g, :],
                            start=(g == 0), stop=(g == NT - 1),
                        )

            # emit pieces in plan (execution) order; deep pieces are
            # contiguous and bundled; strip fold lands after the last deep
            # piece; assembly A runs inside half B
            def emit_mid_consts():
                ident = const.tile([128, 128], f32)
                make_identity(nc, ident[:])
                identb = const.tile([128, 128], bf16)
                nc.vector.tensor_copy(out=identb[:], in_=ident[:])
                w1b = const.tile([D, D], bf16)
                nc.vector.tensor_copy(out=w1b[:], in_=w1s[:])
                ones64 = const.tile([D, 1], f32)
                nc.vector.memset(ones64[:], 1.0)
                b1sq = persist.tile([D, 1], f32, name="b1sq")
                nc.vector.tensor_tensor(out=b1sq[:], in0=b1c[:], in1=b1c[:],
                                        op=mybir.AluOpType.mult)
                nc.vector.memset(aggs[:, :, D : D + 1], 1.0)
                b2_ps = wpsum.tile([NUM_GRAPHS, D], f32, tag="ps_b",
                                   name="b2_ps")
                nc.tensor.matmul(out=b2_ps[:], lhsT=p1s[:], rhs=b2r[:],
                                 start=True, stop=True)
                b2m = persist.tile([NUM_GRAPHS, D], f32, name="b2m")
                nc.vector.tensor_copy(out=b2m[:], in_=b2_ps[:])
                return identb, w1b, ones64, b1sq, b2m

            def emit_fold():
                # t = dinv^2*x_own + deep strip (all D pieces done); runs on
                # DVE while Pool continues the remaining rounds
                nc.vector.tensor_tensor(
                    out=tsl[:], in0=xos[:],
                    in1=d2s[:, :].rearrange(
                        "p (g o) -> p g o", o=1).to_broadcast([128, NT, D]),
                    op=mybir.AluOpType.mult,
                )
                stb = persist.tile([128, NT, D], bf16, name="stb")
                nc.scalar.dma_start(
                    out=stb[:],
                    in_=strip_rows.rearrange("(g p) c -> p g c", p=128)[
                        :, 0:NT, 0:D],
                )
                nc.vector.tensor_tensor(out=tsl[:], in0=tsl[:], in1=stb[:],
                                        op=mybir.AluOpType.add)

            last_d = max((i for i, pc in enumerate(pieces)
                          if pc["stage"] == -1), default=-1)
            first_s2 = next((i for i, pc in enumerate(pieces)
                             if pc["stage"] == 2), len(pieces))
            bundle, bn_tot = [], 0
            folded = False
            consts_done = False
            asm0_done = False
            for i, pc in enumerate(pieces):
                if pc["stage"] == -1:
                    if bn_tot + pc["n"] > SMAXC and bundle:
                        deep_bundle(bundle)
                        bundle, bn_tot = [], 0
                    bundle.append(pc)
                    bn_tot += pc["n"]
                else:
                    if bundle:
                        deep_bundle(bundle)
                        bundle, bn_tot = [], 0
                    edge_piece(pc)
                if i >= last_d and not folded:
                    if bundle:
                        deep_bundle(bundle)
                        bundle, bn_tot = [], 0
                    emit_fold()
                    folded = True
                if i >= 1 and not consts_done:
                    identb, w1b, ones64, b1sq, b2m = emit_mid_consts()
                    consts_done = True
                if i >= first_s2 and not asm0_done:
                    assemble(0)  # overlaps half-B edge work
                    asm0_done = True
            if bundle:
                deep_bundle(bundle)
            if not folded:
                emit_fold()
            if not consts_done:
                identb, w1b, ones64, b1sq, b2m = emit_mid_consts()
            if not asm0_done:
                assemble(0)
            assemble(1)
            stats_sb = persist.tile([D, D + 1], bf16, name="stats_sb")
            nc.scalar.activation(stats_sb[:], stats_ps[:],
                                 mb.ActivationFunctionType.Copy)
            nc.sync.dma_start(out=sag_in[:, :], in_=stats_sb[:])
            nc.gpsimd.collective_compute(
                "AllGather", mybir.AluOpType.bypass, replica_groups=RG,
                ins=[sag_in[:, :]], outs=[sag_out[:, :]],
            )

            # --- transposed h (pre-BN) while the AllGather is in flight ---
            hT_big = persist.tile([D, NT * 128], bf16, name="hT_big")
            for b0 in range(0, NT, 4):
                bn = min(4, NT - b0)
                tp_ps = wpsum.tile([D, 512], bf16, tag="ps_a", name="tp_ps")
                for j in range(bn):
                    b = b0 + j
                    nc.tensor.transpose(
                        out=tp_ps[:, j * 128 : (j + 1) * 128],
                        in_=aggs[:, b, :D], identity=identb[:],
                    )
                aggsT = work.tile([D, 512], bf16, tag="aggsT", name="aggsT",
                                  bufs=2)
                nc.vector.tensor_copy(out=aggsT[:, : bn * 128],
                                      in_=tp_ps[:, : bn * 128])
                hT_ps = wpsum.tile([D, 512], f32, tag="ps_b", name="hT_ps")
                nc.tensor.matmul(
                    out=hT_ps[:, : bn * 128], lhsT=w1b[:],
                    rhs=aggsT[:, : bn * 128], start=True, stop=True,
                )
                nc.scalar.activation(
                    hT_big[:, b0 * 128 : (b0 + bn) * 128],
                    hT_ps[:, : bn * 128], mb.ActivationFunctionType.Copy,
                )

            # --- stats tree-sum + BN scalar algebra ---
            st8 = persist.tile([D, NCORES, D + 1], bf16, name="st8")
            nc.sync.dma_start(
                out=st8[:], in_=sag_out[:, :].rearrange("(r p) c -> p r c",
                                                        p=D)
            )
            st4 = persist.tile([D, 4, D + 1], f32, name="st4")
            nc.vector.tensor_tensor(
                out=st4[:], in0=st8[:, 0:4, :], in1=st8[:, 4:8, :],
                op=mybir.AluOpType.add,
            )
            nc.vector.tensor_tensor(
                out=st4[:, 0:2, :], in0=st4[:, 0:2, :], in1=st4[:, 2:4, :],
                op=mybir.AluOpType.add,
            )
            st = persist.tile([D, D + 1], f32, name="st")
            nc.vector.tensor_tensor(
                out=st[:], in0=st4[:, 0, :], in1=st4[:, 1, :],
                op=mybir.AluOpType.add,
            )

            q_ps = wpsum.tile([D, 1], f32, tag="ps_a", name="q_ps")
            nc.tensor.matmul(out=q_ps[:], lhsT=w1s[:], rhs=st[:, D : D + 1],
                             start=True, stop=True)
            mu = persist.tile([D, 1], f32, name="mu")
            nc.vector.tensor_scalar(
                out=mu[:], in0=q_ps[:], scalar1=1.0 / NN, scalar2=b1c[:],
                op0=mybir.AluOpType.mult, op1=mybir.AluOpType.add,
            )
            t1_ps = wpsum.tile([D, D], f32, tag="ps_b", name="t1_ps")
            nc.tensor.matmul(out=t1_ps[:], lhsT=st[:, :D], rhs=w1s[:],
                             start=True, stop=True)
            m_sb = work.tile([D, D], f32, tag="m_sb", name="m_sb")
            nc.vector.tensor_tensor(out=m_sb[:], in0=w1s[:], in1=t1_ps[:],
                                    op=mybir.AluOpType.mult)
            d_ps = wpsum.tile([D, 1], f32, tag="ps_b", name="d_ps")
            nc.tensor.matmul(out=d_ps[:], lhsT=m_sb[:], rhs=ones64[:],
                             start=True, stop=True)

            var = persist.tile([D, 1], f32, name="var")
            t2 = work.tile([D, 1], f32, tag="t2", name="t2")
            nc.vector.tensor_scalar(
                out=t2[:], in0=q_ps[:], scalar1=2.0 / NN, scalar2=b1c[:],
                op0=mybir.AluOpType.mult, op1=mybir.AluOpType.mult,
            )
            nc.vector.tensor_scalar(
                out=var[:], in0=d_ps[:], scalar1=1.0 / NN, scalar2=t2[:],
                op0=mybir.AluOpType.mult, op1=mybir.AluOpType.add,
            )
            nc.vector.tensor_tensor(out=var[:], in0=var[:], in1=b1sq[:],
                                    op=mybir.AluOpType.add)
            t4 = work.tile([D, 1], f32, tag="t4", name="t4")
            nc.vector.tensor_tensor(out=t4[:], in0=mu[:], in1=mu[:],
                                    op=mybir.AluOpType.mult)
            nc.vector.tensor_tensor(out=var[:], in0=var[:], in1=t4[:],
                                    op=mybir.AluOpType.subtract)

            sd = work.tile([D, 1], f32, tag="sd", name="sd")
            nc.scalar.activation(sd[:], var[:], mb.ActivationFunctionType.Sqrt,
                                 bias=epsc[:])
            rstd = work.tile([D, 1], f32, tag="rstd", name="rstd")
            nc.vector.reciprocal(out=rstd[:], in_=sd[:])
            a_sb = persist.tile([D, 1], f32, name="a_sb")
            nc.vector.tensor_tensor(out=a_sb[:], in0=gac[:], in1=rstd[:],
                                    op=mybir.AluOpType.mult)
            c_sb = persist.tile([D, 1], f32, name="c_sb")
            t5 = work.tile([D, 1], f32, tag="t5", name="t5")
            nc.vector.tensor_tensor(out=t5[:], in0=mu[:], in1=a_sb[:],
                                    op=mybir.AluOpType.mult)
            nc.vector.tensor_tensor(out=c_sb[:], in0=bec[:], in1=t5[:],
                                    op=mybir.AluOpType.subtract)
            # hT excludes the b1 bias; fold it into the BN offset
            t6 = work.tile([D, 1], f32, tag="t6", name="t6")
            nc.vector.tensor_tensor(out=t6[:], in0=a_sb[:], in1=b1c[:],
                                    op=mybir.AluOpType.mult)
            nc.vector.tensor_tensor(out=c_sb[:], in0=c_sb[:], in1=t6[:],
                                    op=mybir.AluOpType.add)

            # --- BN+ReLU, transpose back, pool matmul ---
            h1 = persist.tile([128, NT, D], bf16, name="h1")
            poolT_ps = spsum.tile([D, NUM_GRAPHS], f32, name="poolT_ps")
            for b0 in range(0, NT, 4):
                bn = min(4, NT - b0)
                h1T = work.tile([D, 512], bf16, tag="h1T", name="h1T", bufs=2)
                nc.scalar.activation(
                    h1T[:, : bn * 128],
                    hT_big[:, b0 * 128 : (b0 + bn) * 128],
                    mb.ActivationFunctionType.Relu, bias=c_sb[:], scale=a_sb[:],
                )
                for j in range(bn):
                    b = b0 + j
                    nm_ps = wpsum.tile([128, D], bf16, tag="ps_a", name="nm_ps")
                    nc.tensor.transpose(
                        out=nm_ps[:], in_=h1T[:, j * 128 : (j + 1) * 128],
                        identity=identb[:D, :D],
                    )
                    nc.vector.tensor_copy(out=h1[:, b, :], in_=nm_ps[:])
                    nc.tensor.matmul(
                        out=poolT_ps[:], lhsT=h1[:, b, :], rhs=qs[:, b, :],
                        start=(b == 0), stop=(b == NT - 1),
                    )

            # --- out partial, AllGather (bf16), tree sum, +b2, store ---
            poolT_sb = persist.tile([D, NUM_GRAPHS], f32, name="poolT_sb")
            nc.vector.tensor_copy(out=poolT_sb[:], in_=poolT_ps[:])
            out_ps = wpsum.tile([NUM_GRAPHS, D], f32, tag="ps_b", name="out_ps")
            nc.tensor.matmul(out=out_ps[:], lhsT=poolT_sb[:], rhs=w2s[:],
                             start=True, stop=True)
            out_sb = persist.tile([NUM_GRAPHS, D], bf16, name="out_sb")
            nc.vector.tensor_copy(out=out_sb[:], in_=out_ps[:])
            nc.sync.dma_start(out=oag_in[:, :], in_=out_sb[:])
            nc.gpsimd.collective_compute(
                "AllGather", mybir.AluOpType.bypass, replica_groups=RG,
                ins=[oag_in[:, :]], outs=[oag_out[:, :]],
            )
            o8 = persist.tile([NUM_GRAPHS, NCORES, D], bf16, name="o8")
            nc.sync.dma_start(
                out=o8[:],
                in_=oag_out[:, :].rearrange("(r p) c -> p r c", p=NUM_GRAPHS),
            )
            o4 = persist.tile([NUM_GRAPHS, 4, D], f32, name="o4")
            nc.vector.tensor_tensor(
                out=o4[:], in0=o8[:, 0:4, :], in1=o8[:, 4:8, :],
                op=mybir.AluOpType.add,
            )
            nc.vector.tensor_tensor(
                out=o4[:, 0:2, :], in0=o4[:, 0:2, :], in1=o4[:, 2:4, :],
                op=mybir.AluOpType.add,
            )
            outf = persist.tile([NUM_GRAPHS, D], f32, name="outf")
            nc.vector.tensor_tensor(
                out=outf[:], in0=o4[:, 0, :], in1=o4[:, 1, :],
                op=mybir.AluOpType.add,
            )
            nc.vector.tensor_tensor(
                out=outf[:], in0=outf[:], in1=b2m[:], op=mybir.AluOpType.add,
            )
            nc.sync.dma_start(out=out_d[:, :], in_=outf[:])

    nc.compile()
    return nc


def _wrap16(v, n):
    """idx j at [j%16, j//16], replicated to 128 partitions (8 Q7 cores)."""
    assert v.shape[0] == n and n % 16 == 0
    t = v.astype(np.int16).reshape(n // 16, 16).T
    return np.tile(t, (8, 1))


def _wrap128(v, n):
    """value j at [j%128, j//128] (norm layout for the gather stream)."""
    assert v.shape[0] == n and n % 128 == 0
    return np.ascontiguousarray(v.astype(np.float32).reshape(n // 128, 128).T)


def _up128(v):
    return ((v + 127) // 128) * 128 if v else 0


def _slot_of(l):
    """node local id -> slot (upper half shifted past half-A pad zone)."""
    return np.where(l < HREAL, l, l + HPAD)


def prepare_inputs(cfg, x, edge_index, batch, W1, b1, gamma, beta, W2, b2):
    """Host-side index preprocessing + per-core input maps. Fills cfg.pieces."""
    x = np.ascontiguousarray(np.asarray(x, dtype=np.float32))
    src = np.asarray(edge_index[0], dtype=np.int64)
    dst = np.asarray(edge_index[1], dtype=np.int64)
    batch = np.asarray(batch, dtype=np.int64)
    W1 = np.asarray(W1, dtype=np.float32)
    b1 = np.asarray(b1, dtype=np.float32)
    gamma = np.asarray(gamma, dtype=np.float32)
    beta = np.asarray(beta, dtype=np.float32)
    W2 = np.asarray(W2, dtype=np.float32)
    b2 = np.asarray(b2, dtype=np.float32)
    n = N_NODES

    deg = np.bincount(dst, minlength=n).astype(np.float64) + 1.0  # + self-loop
    dinv = 1.0 / np.sqrt(deg)

    cnt = np.bincount(batch, minlength=NUM_GRAPHS).astype(np.float64)
    w_graph = 1.0 / np.maximum(cnt, 1.0)

    # q pooling matrix for layer 2 (index data only)
    wg = w_graph[batch]
    q = np.bincount(
        src * NUM_GRAPHS + batch[dst],
        weights=dinv[src] * dinv[dst] * wg[dst],
        minlength=n * NUM_GRAPHS,
    )
    q += np.bincount(
        np.arange(n) * NUM_GRAPHS + batch,
        weights=dinv * dinv * wg,
        minlength=n * NUM_GRAPHS,
    )
    q = q.reshape(n, NUM_GRAPHS).astype(np.float32)
    p1 = (cnt > 0).astype(np.float32).reshape(1, NUM_GRAPHS)
    norm_all = (dinv[src] * dinv[dst]).astype(np.float32)
    dinv = dinv.astype(np.float32)

    xr = np.zeros((TROWS, D), dtype=np.float32)
    xr[:n] = x

    import ml_dtypes

    # per-core rounds: (slots, srcs, norms, nA=src<THALF count), A-first order
    per_core = []
    for k in range(NCORES):
        sel = (dst >= k * SL) & (dst < (k + 1) * SL)
        sl_d = _slot_of(dst[sel] - k * SL)
        sr = src[sel]
        nm = norm_all[sel]
        order = np.argsort(sl_d, kind="stable")
        sl_d, sr, nm = sl_d[order], sr[order], nm[order]
        chg = np.r_[True, sl_d[1:] != sl_d[:-1]] if len(sl_d) else np.zeros(
            0, bool)
        starts = np.flatnonzero(chg)
        gg = np.cumsum(chg) - 1
        occ = (np.arange(len(sl_d)) - starts[gg]) if len(sl_d) else np.zeros(
            0, np.int64)
        nr = int(occ.max()) + 1 if len(occ) else 0
        rounds = []
        for r in range(nr):
            m = occ == r
            rounds.append((sl_d[m], sr[m], nm[m]))
        per_core.append(rounds)
    nr_max = max(len(rc) for rc in per_core)

    def parts(rc, r, half_sel):
        if r >= len(rc):
            e = np.zeros(0, np.int64)
            return e, e, np.zeros(0, np.float32), 0
        sl_d, sr, nm = rc[r]
        if half_sel is not None:
            m = (sl_d // HALF_S) == half_sel
            sl_d, sr, nm = sl_d[m], sr[m], nm[m]
        ha = sr < THALF
        return (np.r_[sl_d[ha], sl_d[~ha]], np.r_[sr[ha], sr[~ha]],
                np.r_[nm[ha], nm[~ha]].astype(np.float32), int(ha.sum()))

    # piece plan (identical instruction shapes on every core), in EXECUTION
    # order so the chunked idx loads stream in the order pieces consume them.
    # Small cross-half (stage 0) and deep (stage -1) pieces are interleaved
    # among the first half-A rounds: their DVE norm-mults hide under the big
    # rounds' Pool gather/scatter time instead of serializing at the start.
    p_mid, p_deep, p_a, p_b = [], [], [], []
    for r in range(nr_max):
        nmax = max(len(rc[r][0]) if r < len(rc) else 0 for rc in per_core)
        if nmax >= 2 * SBUF_MIN:
            p_a.append(("S", r, 0, 1))
            p_b.append(("S", r, 1, 2))
        elif nmax >= SBUF_MIN:
            p_mid.append(("S", r, None, 0))
        else:
            p_deep.append(("D", r, None, -1))
    plans = []
    ins_a = list(p_a)
    plans.append(ins_a.pop(0)) if ins_a else None
    for m in p_mid:
        plans.append(m)
        if ins_a:
            plans.append(ins_a.pop(0))
    plans.extend(p_deep)  # contiguous: device bundles them
    plans.extend(ins_a)
    plans.extend(p_b)

    cfg.pieces = []
    core_arrs = [([], [], []) for _ in range(NCORES)]
    off = 0
    for kind, r, half_sel, stage in plans:
        nA = _up128(max(parts(rc, r, half_sel)[3] for rc in per_core))
        nB = _up128(max(len(parts(rc, r, half_sel)[0])
                        - parts(rc, r, half_sel)[3] for rc in per_core))
        ntot = nA + nB
        if ntot == 0:
            continue
        if kind == "S":
            cols = (0, HALF_G // 2) if half_sel == 0 else (
                (HALF_G // 2, HALF_G) if half_sel == 1 else (0, HALF_G))
        else:
            cols = None
        cfg.pieces.append(
            {"kind": kind, "n": ntot, "nA": nA, "cols": cols, "stage": stage,
             "goff": off // 16, "soff": off // 16, "noff": off // 128,
             "rnd": r}
        )
        for k in range(NCORES):
            sl_d, sr, nm, nAr = parts(per_core[k], r, half_sel)
            nBr = len(sl_d) - nAr
            ga = np.zeros(ntot, dtype=np.int64)
            na = np.zeros(ntot, dtype=np.float32)
            sa = np.zeros(ntot, dtype=np.int64)
            ga[:nAr] = sr[:nAr]
            na[:nAr] = nm[:nAr]
            ga[nA : nA + nBr] = sr[nAr:] - THALF
            na[nA : nA + nBr] = nm[nAr:]
            pad_pos = np.r_[np.arange(nAr, nA), np.arange(nA + nBr, ntot)]
            if kind == "S":
                c0 = cols[0]

                def enc(s):
                    p = s % 128
                    g = s // 128
                    return (((g >> 1) - c0) * 2 + (g & 1)) * 128 + p

                sa[:nAr] = enc(sl_d[:nAr])
                sa[nA : nA + nBr] = enc(sl_d[nAr:])
                if len(pad_pos):
                    # dedicated pad slots inside this piece's col range
                    pad_base = HREAL if cols[0] == 0 else HALF_S + HREAL
                    avail = HPAD
                    assert len(pad_pos) <= avail, (len(pad_pos), avail)
                    sa[pad_pos] = enc(pad_base + np.arange(len(pad_pos)))
            else:
                sa[:nAr] = sl_d[:nAr]
                sa[nA : nA + nBr] = sl_d[nAr:]
                assert len(pad_pos) <= STRIP_ROWS - SLP
                sa[pad_pos] = SLP + np.arange(len(pad_pos))
            core_arrs[k][0].append(_wrap16(ga, ntot))
            core_arrs[k][1].append(_wrap16(sa, ntot))
            core_arrs[k][2].append(_wrap128(na, ntot))
        off += ntot

    # per-core node-major own arrays (slot layout)
    in_maps = []
    ll = np.arange(SL)
    ss = _slot_of(ll)
    P_arr = ss % 128
    G_arr = ss // 128
    for k in range(NCORES):
        lo = k * SL
        xo = np.zeros((128, NT, D), dtype=np.float32)
        xo[P_arr, G_arr] = x[lo : lo + SL]
        qslc = np.zeros((128, NT, NUM_GRAPHS), dtype=np.float32)
        qslc[P_arr, G_arr] = q[lo : lo + SL]
        d2 = np.zeros((128, NT), dtype=np.float32)
        d2[P_arr, G_arr] = dinv[lo : lo + SL] ** 2
        im = {
            "xr": xr,
            "xo": np.ascontiguousarray(xo.reshape(128, NT * D)),
            "d2o": d2,
            "qsl": np.ascontiguousarray(
                qslc.reshape(128, NT * NUM_GRAPHS).astype(ml_dtypes.bfloat16)),
            "glo": np.concatenate(core_arrs[k][0], axis=1),
            "sct": np.concatenate(core_arrs[k][1], axis=1),
            "nrm": np.concatenate(core_arrs[k][2], axis=1),
            "p1": p1,
            "w1": W1,
            "b1": b1.reshape(D, 1),
            "ga": gamma.reshape(D, 1),
            "be": beta.reshape(D, 1),
            "w2": W2,
            "b2": b2.reshape(1, D),
        }
        in_maps.append(im)
    return in_maps


def kernel(x, edge_index, batch, W1, b1, gamma, beta, W2, b2):
    global LAST_EXEC_TIME_NS, _LAST_IN_MAPS
    from concourse.bass_utils import run_bass_kernel_spmd

    cfg = Cfg()
    in_maps = prepare_inputs(cfg, x, edge_index, batch, W1, b1, gamma, beta,
                             W2, b2)

    key = tuple((p["kind"], p["n"], p["nA"], p["cols"], p["stage"])
                for p in cfg.pieces)
    if key not in _NC_CACHE:
        _NC_CACHE[key] = build(cfg)
    nc = _NC_CACHE[key]
    _LAST_IN_MAPS = in_maps

    res = run_bass_kernel_spmd(nc, in_maps, list(range(NCORES)), trace=False)
    LAST_EXEC_TIME_NS = res.exec_time_ns
    return np.asarray(res.results[0]["out"], dtype=np.float32)


def modeled_time_ns(**kw):
    """Cost-model execution time (MultiCoreSim, mocked collectives)."""
    if not _NC_CACHE:
        return None
    nc = next(iter(_NC_CACHE.values()))
    ins = _LAST_IN_MAPS
    if ins is None:
        return None
    from concourse.bass_interp import MultiCoreSim

    sim = MultiCoreSim(nc, 2, debug_mock_collectives_without_correctness=True)
    for i, core in sim.cores.items():
        for name, val in ins[i].items():
            core.tensor(name)[:] = val
    sim.simulate()
    return int(sim.global_time)


# revision 4
# speedup vs baseline: 1.0008x; 1.0008x over previous
# GCN (2-layer GCNConv + BatchNorm + ReLU + global mean pool) on 8 TRN2 cores.
#
# v4 strategy (dst-partitioned, no ReduceScatter):
#   * Edges partitioned by DST owner.  Every core holds the FULL raw-x row
#     table (host-replicated, pure permutation/padding) in DRAM, split in two
#     25088-row halves so int16 gather indices stay in range.
#   * Core k owns nodes [k*6250,(k+1)*6250); node l -> slot s (s = l, shifted
#     +459 for the upper shard half so each half owns 459 dedicated pad
#     slots); slot s -> partition s%128, group s//128 (56 groups, 28 pairs).
#   * Per-dst-occurrence rounds keep scatter rows unique per instruction.
#     Pipeline per round: Pool gather (f32, src-half substreams joined in one
#     buffer) -> DVE norm-mult (msg * norm_e broadcast over channels, bf16
#     out; norm = dinv[src]*dinv[dst] from index data, 0 on pads) -> Pool
#     scatter-add into an SBUF even/odd-group accumulator pair.
#   * Rounds big enough are split into shard halves A (groups 0..27) and
#     B (28..55); all A pieces run before B so A's aggs assembly + BN-stats
#     matmuls overlap B's edge work.  Mid rounds run unsplit (early).  Deep
#     rounds (small) scatter-add onto a zeroed DRAM strip (per-index cost, no
#     SBUF footprint floor) and fold in via one reload+add before assembly.
#   * aggs = acc + (dinv^2 * x_own + strip);  BN stats via the accumulated
#     A^T[A|1] matmul; cross-core stats via bf16 AllGather + on-chip tree
#     sum; one more bf16 AllGather for the pooled output partials.
#   * Layer 2 never materializes per-node features: out = (q . h1) W2 + b2
#     with q[u,g] built on the host from INDEX data only.
#
# Host-side preprocessing uses only index data (edge_index, batch): degrees,
# edge partitioning/rounds, per-edge norms, the q pooling matrix.  Feature
# data (x) is only permuted/padded on the host, never computed with.

import numpy as np

N_NODES = 50000
N_EDGES = 800000
D = 64
NCORES = 8
NUM_GRAPHS = 64
BN_EPS = 1e-5
SL = N_NODES // NCORES          # 6250 owned nodes per core
NT = 56                         # groups
SLP = 128 * NT                  # 7168 padded slots
HALF_G = NT // 2                # 28 groups per shard half
HALF_S = SLP // 2               # 3584 slots per half
HREAL = SL // 2                 # 3125 real nodes per half
HPAD = HALF_S - HREAL           # 459 pad slots per half
TROWS = 50176                   # padded table rows (2 * 25088)
THALF = TROWS // 2              # 25088 (int16-safe)
STRIP_G = NT + 3                # strip groups (56 real + 3 pad)
STRIP_ROWS = 128 * STRIP_G      # 7552
SBUF_MIN = 1500                 # min real rows for the SBUF scatter path


class Cfg:
    def __init__(self):
        # pieces: execution-ordered dicts
        #  {'kind':'S'|'D', 'n', 'nA', 'nB', 'cols':(c0,c1) pair-col slice,
        #   'stage':0 cross/1 halfA/2 halfB/-1 deep, 'goff','soff','noff'}
        self.pieces = []


LAST_EXEC_TIME_NS = None
_NC_CACHE = {}
_LAST_IN_MAPS = None


def build(cfg):
    import concourse.mybir as mybir
    import concourse.mybir as mb
    import concourse.tile as tile
    from concourse import bacc
    from concourse.masks import make_identity

    f32 = mybir.dt.float32
    bf16 = mybir.dt.bfloat16
    i16 = mybir.dt.int16
    NN = float(N_NODES)
    RG = [list(range(NCORES))]
    pieces = cfg.pieces
    tot_s = sum(p["n"] for p in pieces)
    SMAXC = max(p["n"] for p in pieces)

    nc = bacc.Bacc(
        "TRN2", target_bir_lowering=False, debug=False, num_devices=NCORES
    )

    # --- external inputs ---
    xr = nc.declare_dram_parameter("xr", [TROWS, D], f32, isOutput=False)
    xo = nc.declare_dram_parameter("xo", [128, NT * D], f32, isOutput=False)
    d2o = nc.declare_dram_parameter("d2o", [128, NT], f32, isOutput=False)
    qsl = nc.declare_dram_parameter("qsl", [128, NT * NUM_GRAPHS], bf16,
                                    isOutput=False)
    glo_d = nc.declare_dram_parameter("glo", [128, tot_s // 16], i16,
                                      isOutput=False)
    sct_d = nc.declare_dram_parameter("sct", [128, tot_s // 16], i16,
                                      isOutput=False)
    nrm_d = nc.declare_dram_parameter("nrm", [128, tot_s // 128], f32,
                                      isOutput=False)
    p1_d = nc.declare_dram_parameter("p1", [1, NUM_GRAPHS], f32, isOutput=False)
    w1_d = nc.declare_dram_parameter("w1", [D, D], f32, isOutput=False)
    b1_d = nc.declare_dram_parameter("b1", [D, 1], f32, isOutput=False)
    ga_d = nc.declare_dram_parameter("ga", [D, 1], f32, isOutput=False)
    be_d = nc.declare_dram_parameter("be", [D, 1], f32, isOutput=False)
    w2_d = nc.declare_dram_parameter("w2", [D, D], f32, isOutput=False)
    b2_d = nc.declare_dram_parameter("b2", [1, D], f32, isOutput=False)
    out_d = nc.declare_dram_parameter("out", [NUM_GRAPHS, D], f32,
                                      isOutput=True)

    # --- internal DRAM ---
    strip = nc.dram_tensor("strip", [STRIP_ROWS, 2 * D], bf16)
    sag_in = nc.dram_tensor("sag_in", [D, D + 1], bf16)
    sag_out = nc.dram_tensor("sag_out", [NCORES * D, D + 1], bf16,
                             addr_space="Shared")
    oag_in = nc.dram_tensor("oag_in", [NUM_GRAPHS, D], bf16)
    oag_out = nc.dram_tensor("oag_out", [NCORES * NUM_GRAPHS, D], bf16,
                             addr_space="Shared")

    xr_rows = xr[:, :]
    strip_rows = strip[:, :]

    with tile.TileContext(nc) as tc:
        with (
            tc.tile_pool(name="const", bufs=1) as const,
            tc.tile_pool(name="persist", bufs=1) as persist,
            tc.tile_pool(name="work", bufs=3) as work,
            tc.tile_pool(name="idxp", bufs=3) as idxp,
            tc.tile_pool(name="msgp", bufs=3) as msgp,
            tc.tile_pool(name="msgbp", bufs=3) as msgbp,
            tc.tile_pool(name="spsum", bufs=1, space="PSUM") as spsum,
            tc.tile_pool(name="wpsum", bufs=3, space="PSUM") as wpsum,
        ):
            # --- accumulators + zeroed strip (first: unblock stage-0/deep) ---
            own = persist.tile([128, HALF_G, D], bf16, name="own")    # even g
            peer = persist.tile([128, HALF_G, D], bf16, name="peer")  # odd g
            nc.vector.memset(own[:], 0.0)
            nc.vector.memset(peer[:], 0.0)
            zs = persist.tile([128, 8, 2 * D], bf16, name="zs")
            nc.vector.memset(zs[:], 0.0)
            epsc = const.tile([D, 1], f32)
            nc.vector.memset(epsc[:], BN_EPS)
            strip_v = strip_rows.rearrange("(g p) c -> p g c", p=128)
            for g0 in range(0, STRIP_G, 8):
                gn = min(8, STRIP_G - g0)
                nc.scalar.dma_start(out=strip_v[:, g0 : g0 + gn, :],
                                    in_=zs[:, :gn, :])
            # preload the Sqrt/Relu act tables off the critical path
            warm = const.tile([1, 1], f32)
            nc.scalar.activation(warm[:], epsc[0:1, :],
                                 mb.ActivationFunctionType.Sqrt)
            nc.scalar.activation(warm[:], epsc[0:1, :],
                                 mb.ActivationFunctionType.Relu)

            # --- batched idx/norm loads (chunked, execution order); chunk 0
            # on SP covers the whole stage-0 + deep prefix ---
            glo_s = persist.tile([128, tot_s // 16], i16, name="glo_s")
            sct_s = persist.tile([128, tot_s // 16], i16, name="sct_s")
            nrm_s = persist.tile([128, tot_s // 128], f32, name="nrm_s")
            ICHUNK = 1792
            bounds = [0, 256] + list(range(ICHUNK, tot_s // 16, ICHUNK)) + [
                tot_s // 16]
            for qi in range(len(bounds) - 1):
                c0, c1 = bounds[qi], bounds[qi + 1]
                if c0 >= c1:
                    continue
                eng = nc.sync if qi % 2 == 0 else nc.scalar
                eng.dma_start(out=glo_s[:, c0:c1], in_=glo_d[:, c0:c1])
                eng.dma_start(out=sct_s[:, c0:c1], in_=sct_d[:, c0:c1])
                d0, d1 = c0 // 8, min(c1 // 8, tot_s // 128)
                if qi == len(bounds) - 2:
                    d1 = tot_s // 128
                if d0 < d1:
                    eng.dma_start(out=nrm_s[:, d0:d1], in_=nrm_d[:, d0:d1])

            # --- constants (scalar queue, after the idx chunks) ---
            w1s = const.tile([D, D], f32)
            nc.scalar.dma_start(out=w1s[:], in_=w1_d[:, :])
            w2s = const.tile([D, D], f32)
            nc.scalar.dma_start(out=w2s[:], in_=w2_d[:, :])
            b1c = const.tile([D, 1], f32)
            nc.scalar.dma_start(out=b1c[:], in_=b1_d[:, :])
            gac = const.tile([D, 1], f32)
            nc.scalar.dma_start(out=gac[:], in_=ga_d[:, :])
            bec = const.tile([D, 1], f32)
            nc.scalar.dma_start(out=bec[:], in_=be_d[:, :])
            b2r = const.tile([1, D], f32)
            nc.scalar.dma_start(out=b2r[:], in_=b2_d[:, :])
            p1s = const.tile([1, NUM_GRAPHS], f32)
            nc.scalar.dma_start(out=p1s[:], in_=p1_d[:, :])
            d2s = const.tile([128, NT], f32)
            nc.scalar.dma_start(out=d2s[:], in_=d2o[:, :])
            xos = persist.tile([128, NT, D], f32, name="xos")
            nc.scalar.dma_start(
                out=xos[:], in_=xo[:, :].rearrange("p (g d) -> p g d", d=D)
            )
            qs = persist.tile([128, NT, NUM_GRAPHS], bf16, name="qs")
            nc.scalar.dma_start(
                out=qs[:], in_=qsl[:, :].rearrange("p (g d) -> p g d",
                                                   d=NUM_GRAPHS)
            )

            # --- edge pipeline ---
            def edge_piece(pc):
                n = pc["n"]
                nA = pc["nA"]
                so = pc["soff"]
                no = pc["noff"]
                msg = msgp.tile([128, SMAXC // 128, D], f32, tag="msg",
                                name="msg")
                for half, go, nseg in ((0, 0, nA), (1, nA, n - nA)):
                    if nseg == 0:
                        continue
                    base = half * THALF
                    nc.gpsimd.dma_gather(
                        out_ap=msg[:, go // 128 : (go + nseg) // 128, :],
                        in_ap=xr_rows[base : base + THALF, :],
                        idxs_ap=glo_s[:, so + go // 16 : so + (go + nseg) // 16],
                        num_idxs=nseg, num_idxs_reg=nseg, elem_size=D,
                        single_packet=False, queue_num=0,
                    )
                msgb = msgbp.tile([128, SMAXC // 128, D], bf16, tag="msgb",
                                  name="msgb")
                nc.vector.tensor_tensor(
                    out=msgb[:, : n // 128, :], in0=msg[:, : n // 128, :],
                    in1=nrm_s[:, no : no + n // 128].rearrange(
                        "p (g o) -> p g o", o=1).to_broadcast(
                        [128, n // 128, D]),
                    op=mybir.AluOpType.mult,
                )
                if pc["kind"] == "S":
                    c0, c1 = pc["cols"]
                    nc.gpsimd.dma_scatter_add(
                        own[:, c0:c1, :], msgb[:, : n // 128, :],
                        sct_s[:, so : so + n // 16], n, n, D,
                        sbuf_tokens_per_rank=128, parity_reg=0,
                        out_ap_other=peer[:, c0:c1, :],
                        single_packet=False, queue_num=0,
                    )
                else:
                    nc.gpsimd.dma_scatter_add(
                        strip_rows[:, 0:D], msgb[:, : n // 128, :],
                        sct_s[:, so : so + n // 16], n, n, D,
                        elem_step=2 * D,
                        single_packet=False, queue_num=0,
                    )

            def deep_bundle(pcs):
                """One gather stream + one norm-mult feeding several per-round
                strip scatters (avoids tiny chained pieces)."""
                n = sum(p["n"] for p in pcs)
                so0 = pcs[0]["soff"]
                no0 = pcs[0]["noff"]
                msg = msgp.tile([128, SMAXC // 128, D], f32, tag="msg",
                                name="msg")
                off = 0
                for pc in pcs:
                    for half, go, nseg in ((0, 0, pc["nA"]),
                                           (1, pc["nA"], pc["n"] - pc["nA"])):
                        if nseg == 0:
                            continue
                        base = half * THALF
                        o = off + go
                        nc.gpsimd.dma_gather(
                            out_ap=msg[:, o // 128 : (o + nseg) // 128, :],
                            in_ap=xr_rows[base : base + THALF, :],
                            idxs_ap=glo_s[:, so0 + o // 16 :
                                          so0 + (o + nseg) // 16],
                            num_idxs=nseg, num_idxs_reg=nseg, elem_size=D,
                            single_packet=False, queue_num=0,
                        )
                    off += pc["n"]
                msgb = msgbp.tile([128, SMAXC // 128, D], bf16, tag="msgb",
                                  name="msgb")
                nc.vector.tensor_tensor(
                    out=msgb[:, : n // 128, :], in0=msg[:, : n // 128, :],
                    in1=nrm_s[:, no0 : no0 + n // 128].rearrange(
                        "p (g o) -> p g o", o=1).to_broadcast(
                        [128, n // 128, D]),
                    op=mybir.AluOpType.mult,
                )
                off = 0
                for pc in pcs:
                    pn = pc["n"]
                    nc.gpsimd.dma_scatter_add(
                        strip_rows[:, 0:D],
                        msgb[:, off // 128 : (off + pn) // 128, :],
                        sct_s[:, so0 + off // 16 : so0 + (off + pn) // 16],
                        pn, pn, D, elem_step=2 * D,
                        single_packet=False, queue_num=0,
                    )
                    off += pn

            aggs = persist.tile([128, NT, D + 1], bf16, name="aggs")
            stats_ps = spsum.tile([D, D + 1], f32, name="stats_ps")
            tsl = persist.tile([128, NT, D], f32, name="tsl")
            agv = aggs[:, :, :D].rearrange("p (a two) d -> p a two d", two=2)
            tsv = tsl[:].rearrange("p (a two) d -> p a two d", two=2)

            def assemble(h):
                # chunked adds + stats matmuls so PE overlaps the DVE adds
                q = HALF_G // 2  # 14 pair-cols per half
                for ci in range(0, q, 4):
                    a0 = h * q + ci
                    a1 = min(a0 + 4, (h + 1) * q)
                    nc.vector.tensor_tensor(
                        out=agv[:, a0:a1, 0, :], in0=own[:, a0:a1, :],
                        in1=tsv[:, a0:a1, 0, :], op=mybir.AluOpType.add,
                    )
                    nc.vector.tensor_tensor(
                        out=agv[:, a0:a1, 1, :], in0=peer[:, a0:a1, :],
                        in1=tsv[:, a0:a1, 1, :], op=mybir.AluOpType.add,
                    )
                    for g in range(2 * a0, 2 * a1):
                        nc.tensor.matmul(
                            out=stats_ps[:], lhsT=aggs[:, g, :D],
                            rhs=aggs[:, g, :],
                            start=(g == 0), stop=(g == NT - 1),
                        )

            # emit pieces in plan (execution) order; deep pieces are
            # contiguous and bundled; strip fold lands after the last deep
            # piece; assembly A runs inside half B
            def emit_mid_consts():
                ident = const.tile([128, 128], f32)
                make_identity(nc, ident[:])
                identb = const.tile([128, 128], bf16)
                nc.vector.tensor_copy(out=identb[:], in_=ident[:])
                w1b = const.tile([D, D], bf16)
                nc.vector.tensor_copy(out=w1b[:], in_=w1s[:])
                ones64 = const.tile([D, 1], f32)
                nc.vector.memset(ones64[:], 1.0)
                b1sq = persist.tile([D, 1], f32, name="b1sq")
                nc.vector.tensor_tensor(out=b1sq[:], in0=b1c[:], in1=b1c[:],
                                        op=mybir.AluOpType.mult)
                nc.vector.memset(aggs[:, :, D : D + 1], 1.0)
                b2_ps = wpsum.tile([NUM_GRAPHS, D], f32, tag="ps_b",
                                   name="b2_ps")
                nc.tensor.matmul(out=b2_ps[:], lhsT=p1s[:], rhs=b2r[:],
                                 start=True, stop=True)
                b2m = persist.tile([NUM_GRAPHS, D], f32, name="b2m")
                nc.vector.tensor_copy(out=b2m[:], in_=b2_ps[:])
                return identb, w1b, ones64, b1sq, b2m

            def emit_fold():
                # t = dinv^2*x_own + deep strip (all D pieces done); runs on
                # DVE while Pool continues the remaining rounds
                nc.vector.tensor_tensor(
                    out=tsl[:], in0=xos[:],
                    in1=d2s[:, :].rearrange(
                        "p (g o) -> p g o", o=1).to_broadcast([128, NT, D]),
                    op=mybir.AluOpType.mult,
                )
                stb = persist.tile([128, NT, D], bf16, name="stb")
                nc.scalar.dma_start(
                    out=stb[:],
                    in_=strip_rows.rearrange("(g p) c -> p g c", p=128)[
                        :, 0:NT, 0:D],
                )
                nc.vector.tensor_tensor(out=tsl[:], in0=tsl[:], in1=stb[:],
                                        op=mybir.AluOpType.add)

            last_d = max((i for i, pc in enumerate(pieces)
                          if pc["stage"] == -1), default=-1)
            first_s2 = next((i for i, pc in enumerate(pieces)
                             if pc["stage"] == 2), len(pieces))
            bundle, bn_tot = [], 0
            folded = False
            consts_done = False
            asm0_done = False
            for i, pc in enumerate(pieces):
                if pc["stage"] == -1:
                    if bn_tot + pc["n"] > SMAXC and bundle:
                        deep_bundle(bundle)
                        bundle, bn_tot = [], 0
                    bundle.append(pc)
                    bn_tot += pc["n"]
                else:
                    if bundle:
                        deep_bundle(bundle)
                        bundle, bn_tot = [], 0
                    edge_piece(pc)
                if i >= last_d and not folded:
                    if bundle:
                        deep_bundle(bundle)
                        bundle, bn_tot = [], 0
                    emit_fold()
                    folded = True
                if i >= 1 and not consts_done:
                    identb, w1b, ones64, b1sq, b2m = emit_mid_consts()
                    consts_done = True
                if i >= first_s2 and not asm0_done:
                    assemble(0)  # overlaps half-B edge work
                    asm0_done = True
            if bundle:
                deep_bundle(bundle)
            if not folded:
                emit_fold()
            if not consts_done:
                identb, w1b, ones64, b1sq, b2m = emit_mid_consts()
            if not asm0_done:
                assemble(0)
            assemble(1)
            stats_sb = persist.tile([D, D + 1], bf16, name="stats_sb")
            nc.scalar.activation(stats_sb[:], stats_ps[:],
                                 mb.ActivationFunctionType.Copy)
            nc.sync.dma_start(out=sag_in[:, :], in_=stats_sb[:])
            nc.gpsimd.collective_compute(
                "AllGather", mybir.AluOpType.bypass, replica_groups=RG,
                ins=[sag_in[:, :]], outs=[sag_out[:, :]],
            )

            # --- transposed h (pre-BN) while the AllGather is in flight ---
            hT_big = persist.tile([D, NT * 128], bf16, name="hT_big")
            for b0 in range(0, NT, 4):
                bn = min(4, NT - b0)
                tp_ps = wpsum.tile([D, 512], bf16, tag="ps_a", name="tp_ps")
                for j in range(bn):
                    b = b0 + j
                    nc.tensor.transpose(
                        out=tp_ps[:, j * 128 : (j + 1) * 128],
                        in_=aggs[:, b, :D], identity=identb[:],
                    )
                aggsT = work.tile([D, 512], bf16, tag="aggsT", name="aggsT",
                                  bufs=2)
                nc.vector.tensor_copy(out=aggsT[:, : bn * 128],
                                      in_=tp_ps[:, : bn * 128])
                hT_ps = wpsum.tile([D, 512], f32, tag="ps_b", name="hT_ps")
                nc.tensor.matmul(
                    out=hT_ps[:, : bn * 128], lhsT=w1b[:],
                    rhs=aggsT[:, : bn * 128], start=True, stop=True,
                )
                nc.scalar.activation(
                    hT_big[:, b0 * 128 : (b0 + bn) * 128],
                    hT_ps[:, : bn * 128], mb.ActivationFunctionType.Copy,
                )

            # --- stats tree-sum + BN scalar algebra ---
            st8 = persist.tile([D, NCORES, D + 1], bf16, name="st8")
            nc.sync.dma_start(
                out=st8[:], in_=sag_out[:, :].rearrange("(r p) c -> p r c",
                                                        p=D)
            )
            st4 = persist.tile([D, 4, D + 1], f32, name="st4")
            nc.vector.tensor_tensor(
                out=st4[:], in0=st8[:, 0:4, :], in1=st8[:, 4:8, :],
                op=mybir.AluOpType.add,
            )
            nc.vector.tensor_tensor(
                out=st4[:, 0:2, :], in0=st4[:, 0:2, :], in1=st4[:, 2:4, :],
                op=mybir.AluOpType.add,
            )
            st = persist.tile([D, D + 1], f32, name="st")
            nc.vector.tensor_tensor(
                out=st[:], in0=st4[:, 0, :], in1=st4[:, 1, :],
                op=mybir.AluOpType.add,
            )

            q_ps = wpsum.tile([D, 1], f32, tag="ps_a", name="q_ps")
            nc.tensor.matmul(out=q_ps[:], lhsT=w1s[:], rhs=st[:, D : D + 1],
                             start=True, stop=True)
            mu = persist.tile([D, 1], f32, name="mu")
            nc.vector.tensor_scalar(
                out=mu[:], in0=q_ps[:], scalar1=1.0 / NN, scalar2=b1c[:],
                op0=mybir.AluOpType.mult, op1=mybir.AluOpType.add,
            )
            t1_ps = wpsum.tile([D, D], f32, tag="ps_b", name="t1_ps")
            nc.tensor.matmul(out=t1_ps[:], lhsT=st[:, :D], rhs=w1s[:],
                             start=True, stop=True)
            m_sb = work.tile([D, D], f32, tag="m_sb", name="m_sb")
            nc.vector.tensor_tensor(out=m_sb[:], in0=w1s[:], in1=t1_ps[:],
                                    op=mybir.AluOpType.mult)
            d_ps = wpsum.tile([D, 1], f32, tag="ps_b", name="d_ps")
            nc.tensor.matmul(out=d_ps[:], lhsT=m_sb[:], rhs=ones64[:],
                             start=True, stop=True)

            var = persist.tile([D, 1], f32, name="var")
            t2 = work.tile([D, 1], f32, tag="t2", name="t2")
            nc.vector.tensor_scalar(
                out=t2[:], in0=q_ps[:], scalar1=2.0 / NN, scalar2=b1c[:],
                op0=mybir.AluOpType.mult, op1=mybir.AluOpType.mult,
            )
            nc.vector.tensor_scalar(
                out=var[:], in0=d_ps[:], scalar1=1.0 / NN, scalar2=t2[:],
                op0=mybir.AluOpType.mult, op1=mybir.AluOpType.add,
            )
            nc.vector.tensor_tensor(out=var[:], in0=var[:], in1=b1sq[:],
                                    op=mybir.AluOpType.add)
            t4 = work.tile([D, 1], f32, tag="t4", name="t4")
            nc.vector.tensor_tensor(out=t4[:], in0=mu[:], in1=mu[:],
                                    op=mybir.AluOpType.mult)
            nc.vector.tensor_tensor(out=var[:], in0=var[:], in1=t4[:],
                                    op=mybir.AluOpType.subtract)

            sd = work.tile([D, 1], f32, tag="sd", name="sd")
            nc.scalar.activation(sd[:], var[:], mb.ActivationFunctionType.Sqrt,
                                 bias=epsc[:])
            rstd = work.tile([D, 1], f32, tag="rstd", name="rstd")
            nc.vector.reciprocal(out=rstd[:], in_=sd[:])
            a_sb = persist.tile([D, 1], f32, name="a_sb")
            nc.vector.tensor_tensor(out=a_sb[:], in0=gac[:], in1=rstd[:],
                                    op=mybir.AluOpType.mult)
            c_sb = persist.tile([D, 1], f32, name="c_sb")
            t5 = work.tile([D, 1], f32, tag="t5", name="t5")
            nc.vector.tensor_tensor(out=t5[:], in0=mu[:], in1=a_sb[:],
                                    op=mybir.AluOpType.mult)
            nc.vector.tensor_tensor(out=c_sb[:], in0=bec[:], in1=t5[:],
                                    op=mybir.AluOpType.subtract)
            # hT excludes the b1 bias; fold it into the BN offset
            t6 = work.tile([D, 1], f32, tag="t6", name="t6")
            nc.vector.tensor_tensor(out=t6[:], in0=a_sb[:], in1=b1c[:],
                                    op=mybir.AluOpType.mult)
            nc.vector.tensor_tensor(out=c_sb[:], in0=c_sb[:], in1=t6[:],
                                    op=mybir.AluOpType.add)

            # --- BN+ReLU, transpose back, pool matmul ---
            h1 = persist.tile([128, NT, D], bf16, name="h1")
            poolT_ps = spsum.tile([D, NUM_GRAPHS], f32, name="poolT_ps")
            for b0 in range(0, NT, 4):
                bn = min(4, NT - b0)
                h1T = work.tile([D, 512], bf16, tag="h1T", name="h1T", bufs=2)
                nc.scalar.activation(
                    h1T[:, : bn * 128],
                    hT_big[:, b0 * 128 : (b0 + bn) * 128],
                    mb.ActivationFunctionType.Relu, bias=c_sb[:], scale=a_sb[:],
                )
                for j in range(bn):
                    b = b0 + j
                    nm_ps = wpsum.tile([128, D], bf16, tag="ps_a", name="nm_ps")
                    nc.tensor.transpose(
                        out=nm_ps[:], in_=h1T[:, j * 128 : (j + 1) * 128],
                        identity=identb[:D, :D],
                    )
                    nc.vector.tensor_copy(out=h1[:, b, :], in_=nm_ps[:])
                    nc.tensor.matmul(
                        out=poolT_ps[:], lhsT=h1[:, b, :], rhs=qs[:, b, :],
                        start=(b == 0), stop=(b == NT - 1),
                    )

            # --- out partial, AllGather (bf16), tree sum, +b2, store ---
            poolT_sb = persist.tile([D, NUM_GRAPHS], f32, name="poolT_sb")
            nc.vector.tensor_copy(out=poolT_sb[:], in_=poolT_ps[:])
            out_ps = wpsum.tile([NUM_GRAPHS, D], f32, tag="ps_b", name="out_ps")
            nc.tensor.matmul(out=out_ps[:], lhsT=poolT_sb[:], rhs=w2s[:],
                             start=True, stop=True)
            out_sb = persist.tile([NUM_GRAPHS, D], bf16, name="out_sb")
            nc.vector.tensor_tensor(out=out_sb[:], in0=out_ps[:], in1=b2m[:],
                                    op=mybir.AluOpType.add)
            nc.sync.dma_start(out=oag_in[:, :], in_=out_sb[:])
            nc.gpsimd.collective_compute(
                "AllGather", mybir.AluOpType.bypass, replica_groups=RG,
                ins=[oag_in[:, :]], outs=[oag_out[:, :]],
            )
            o8 = persist.tile([NUM_GRAPHS, NCORES, D], bf16, name="o8")
            nc.sync.dma_start(
                out=o8[:],
                in_=oag_out[:, :].rearrange("(r p) c -> p r c", p=NUM_GRAPHS),
            )
            o4 = persist.tile([NUM_GRAPHS, 4, D], f32, name="o4")
            nc.vector.tensor_tensor(
                out=o4[:], in0=o8[:, 0:4, :], in1=o8[:, 4:8, :],
                op=mybir.AluOpType.add,
            )
            nc.vector.tensor_tensor(
                out=o4[:, 0:2, :], in0=o4[:, 0:2, :], in1=o4[:, 2:4, :],
                op=mybir.AluOpType.add,
            )
            outf = persist.tile([NUM_GRAPHS, D], f32, name="outf")
            nc.vector.tensor_tensor(
                out=outf[:], in0=o4[:, 0, :], in1=o4[:, 1, :],
                op=mybir.AluOpType.add,
            )
            nc.sync.dma_start(out=out_d[:, :], in_=outf[:])

    nc.compile()
    return nc


def _wrap16(v, n):
    """idx j at [j%16, j//16], replicated to 128 partitions (8 Q7 cores)."""
    assert v.shape[0] == n and n % 16 == 0
    t = v.astype(np.int16).reshape(n // 16, 16).T
    return np.tile(t, (8, 1))


def _wrap128(v, n):
    """value j at [j%128, j//128] (norm layout for the gather stream)."""
    assert v.shape[0] == n and n % 128 == 0
    return np.ascontiguousarray(v.astype(np.float32).reshape(n // 128, 128).T)


def _up128(v):
    return ((v + 127) // 128) * 128 if v else 0


def _slot_of(l):
    """node local id -> slot (upper half shifted past half-A pad zone)."""
    return np.where(l < HREAL, l, l + HPAD)


def prepare_inputs(cfg, x, edge_index, batch, W1, b1, gamma, beta, W2, b2):
    """Host-side index preprocessing + per-core input maps. Fills cfg.pieces."""
    x = np.ascontiguousarray(np.asarray(x, dtype=np.float32))
    src = np.asarray(edge_index[0], dtype=np.int64)
    dst = np.asarray(edge_index[1], dtype=np.int64)
    batch = np.asarray(batch, dtype=np.int64)
    W1 = np.asarray(W1, dtype=np.float32)
    b1 = np.asarray(b1, dtype=np.float32)
    gamma = np.asarray(gamma, dtype=np.float32)
    beta = np.asarray(beta, dtype=np.float32)
    W2 = np.asarray(W2, dtype=np.float32)
    b2 = np.asarray(b2, dtype=np.float32)
    n = N_NODES

    deg = np.bincount(dst, minlength=n).astype(np.float64) + 1.0  # + self-loop
    dinv = 1.0 / np.sqrt(deg)

    cnt = np.bincount(batch, minlength=NUM_GRAPHS).astype(np.float64)
    w_graph = 1.0 / np.maximum(cnt, 1.0)

    # q pooling matrix for layer 2 (index data only)
    wg = w_graph[batch]
    q = np.bincount(
        src * NUM_GRAPHS + batch[dst],
        weights=dinv[src] * dinv[dst] * wg[dst],
        minlength=n * NUM_GRAPHS,
    )
    q += np.bincount(
        np.arange(n) * NUM_GRAPHS + batch,
        weights=dinv * dinv * wg,
        minlength=n * NUM_GRAPHS,
    )
    q = q.reshape(n, NUM_GRAPHS).astype(np.float32)
    # scaled by 1/NCORES: every core folds b2m/8 into its partial before the
    # AllGather, so the summed partials already contain the full b2 bias
    p1 = ((cnt > 0).astype(np.float32) / NCORES).reshape(1, NUM_GRAPHS)
    norm_all = (dinv[src] * dinv[dst]).astype(np.float32)
    dinv = dinv.astype(np.float32)

    xr = np.zeros((TROWS, D), dtype=np.float32)
    xr[:n] = x

    import ml_dtypes

    # per-core rounds: (slots, srcs, norms, nA=src<THALF count), A-first order
    per_core = []
    for k in range(NCORES):
        sel = (dst >= k * SL) & (dst < (k + 1) * SL)
        sl_d = _slot_of(dst[sel] - k * SL)
        sr = src[sel]
        nm = norm_all[sel]
        order = np.argsort(sl_d, kind="stable")
        sl_d, sr, nm = sl_d[order], sr[order], nm[order]
        chg = np.r_[True, sl_d[1:] != sl_d[:-1]] if len(sl_d) else np.zeros(
            0, bool)
        starts = np.flatnonzero(chg)
        gg = np.cumsum(chg) - 1
        occ = (np.arange(len(sl_d)) - starts[gg]) if len(sl_d) else np.zeros(
            0, np.int64)
        nr = int(occ.max()) + 1 if len(occ) else 0
        rounds = []
        for r in range(nr):
            m = occ == r
            rounds.append((sl_d[m], sr[m], nm[m]))
        per_core.append(rounds)
    nr_max = max(len(rc) for rc in per_core)

    def parts(rc, r, half_sel):
        if r >= len(rc):
            e = np.zeros(0, np.int64)
            return e, e, np.zeros(0, np.float32), 0
        sl_d, sr, nm = rc[r]
        if half_sel is not None:
            m = (sl_d // HALF_S) == half_sel
            sl_d, sr, nm = sl_d[m], sr[m], nm[m]
        ha = sr < THALF
        return (np.r_[sl_d[ha], sl_d[~ha]], np.r_[sr[ha], sr[~ha]],
                np.r_[nm[ha], nm[~ha]].astype(np.float32), int(ha.sum()))

    # piece plan (identical instruction shapes on every core), in EXECUTION
    # order so the chunked idx loads stream in the order pieces consume them.
    # Small cross-half (stage 0) and deep (stage -1) pieces are interleaved
    # among the first half-A rounds: their DVE norm-mults hide under the big
    # rounds' Pool gather/scatter time instead of serializing at the start.
    p_mid, p_deep, p_a, p_b = [], [], [], []
    for r in range(nr_max):
        nmax = max(len(rc[r][0]) if r < len(rc) else 0 for rc in per_core)
        if nmax >= 2 * SBUF_MIN:
            p_a.append(("S", r, 0, 1))
            p_b.append(("S", r, 1, 2))
        elif nmax >= SBUF_MIN:
            p_mid.append(("S", r, None, 0))
        else:
            p_deep.append(("D", r, None, -1))
    plans = []
    ins_a = list(p_a)
    plans.append(ins_a.pop(0)) if ins_a else None
    for m in p_mid:
        plans.append(m)
        if ins_a:
            plans.append(ins_a.pop(0))
    plans.extend(p_deep)  # contiguous: device bundles them
    plans.extend(ins_a)
    plans.extend(p_b)

    cfg.pieces = []
    core_arrs = [([], [], []) for _ in range(NCORES)]
    off = 0
    for kind, r, half_sel, stage in plans:
        nA = _up128(max(parts(rc, r, half_sel)[3] for rc in per_core))
        nB = _up128(max(len(parts(rc, r, half_sel)[0])
                        - parts(rc, r, half_sel)[3] for rc in per_core))
        ntot = nA + nB
        if ntot == 0:
            continue
        if kind == "S":
            cols = (0, HALF_G // 2) if half_sel == 0 else (
                (HALF_G // 2, HALF_G) if half_sel == 1 else (0, HALF_G))
        else:
            cols = None
        cfg.pieces.append(
            {"kind": kind, "n": ntot, "nA": nA, "cols": cols, "stage": stage,
             "goff": off // 16, "soff": off // 16, "noff": off // 128,
             "rnd": r}
        )
        for k in range(NCORES):
            sl_d, sr, nm, nAr = parts(per_core[k], r, half_sel)
            nBr = len(sl_d) - nAr
            ga = np.zeros(ntot, dtype=np.int64)
            na = np.zeros(ntot, dtype=np.float32)
            sa = np.zeros(ntot, dtype=np.int64)
            ga[:nAr] = sr[:nAr]
            na[:nAr] = nm[:nAr]
            ga[nA : nA + nBr] = sr[nAr:] - THALF
            na[nA : nA + nBr] = nm[nAr:]
            pad_pos = np.r_[np.arange(nAr, nA), np.arange(nA + nBr, ntot)]
            if kind == "S":
                c0 = cols[0]

                def enc(s):
                    p = s % 128
                    g = s // 128
                    return (((g >> 1) - c0) * 2 + (g & 1)) * 128 + p

                sa[:nAr] = enc(sl_d[:nAr])
                sa[nA : nA + nBr] = enc(sl_d[nAr:])
                if len(pad_pos):
                    # dedicated pad slots inside this piece's col range
                    pad_base = HREAL if cols[0] == 0 else HALF_S + HREAL
                    avail = HPAD
                    assert len(pad_pos) <= avail, (len(pad_pos), avail)
                    sa[pad_pos] = enc(pad_base + np.arange(len(pad_pos)))
            else:
                sa[:nAr] = sl_d[:nAr]
                sa[nA : nA + nBr] = sl_d[nAr:]
                assert len(pad_pos) <= STRIP_ROWS - SLP
                sa[pad_pos] = SLP + np.arange(len(pad_pos))
            core_arrs[k][0].append(_wrap16(ga, ntot))
            core_arrs[k][1].append(_wrap16(sa, ntot))
            core_arrs[k][2].append(_wrap128(na, ntot))
        off += ntot

    # per-core node-major own arrays (slot layout)
    in_maps = []
    ll = np.arange(SL)
    ss = _slot_of(ll)
    P_arr = ss % 128
    G_arr = ss // 128
    for k in range(NCORES):
        lo = k * SL
        xo = np.zeros((128, NT, D), dtype=np.float32)
        xo[P_arr, G_arr] = x[lo : lo + SL]
        qslc = np.zeros((128, NT, NUM_GRAPHS), dtype=np.float32)
        qslc[P_arr, G_arr] = q[lo : lo + SL]
        d2 = np.zeros((128, NT), dtype=np.float32)
        d2[P_arr, G_arr] = dinv[lo : lo + SL] ** 2
        im = {
            "xr": xr,
            "xo": np.ascontiguousarray(xo.reshape(128, NT * D)),
            "d2o": d2,
            "qsl": np.ascontiguousarray(
                qslc.reshape(128, NT * NUM_GRAPHS).astype(ml_dtypes.bfloat16)),
            "glo": np.concatenate(core_arrs[k][0], axis=1),
            "sct": np.concatenate(core_arrs[k][1], axis=1),
            "nrm": np.concatenate(core_arrs[k][2], axis=1),
            "p1": p1,
            "w1": W1,
            "b1": b1.reshape(D, 1),
            "ga": gamma.reshape(D, 1),
            "be": beta.reshape(D, 1),
            "w2": W2,
            "b2": b2.reshape(1, D),
        }
        in_maps.append(im)
    return in_maps


def kernel(x, edge_index, batch, W1, b1, gamma, beta, W2, b2):
    global LAST_EXEC_TIME_NS, _LAST_IN_MAPS
    from concourse.bass_utils import run_bass_kernel_spmd

    cfg = Cfg()
    in_maps = prepare_inputs(cfg, x, edge_index, batch, W1, b1, gamma, beta,
                             W2, b2)

    key = tuple((p["kind"], p["n"], p["nA"], p["cols"], p["stage"])
                for p in cfg.pieces)
    if key not in _NC_CACHE:
        _NC_CACHE[key] = build(cfg)
    nc = _NC_CACHE[key]
    _LAST_IN_MAPS = in_maps

    res = run_bass_kernel_spmd(nc, in_maps, list(range(NCORES)), trace=False)
    LAST_EXEC_TIME_NS = res.exec_time_ns
    return np.asarray(res.results[0]["out"], dtype=np.float32)


def modeled_time_ns(**kw):
    """Cost-model execution time (MultiCoreSim, mocked collectives)."""
    if not _NC_CACHE:
        return None
    nc = next(iter(_NC_CACHE.values()))
    ins = _LAST_IN_MAPS
    if ins is None:
        return None
    from concourse.bass_interp import MultiCoreSim

    sim = MultiCoreSim(nc, 2, debug_mock_collectives_without_correctness=True)
    for i, core in sim.cores.items():
        for name, val in ins[i].items():
            core.tensor(name)[:] = val
    sim.simulate()
    return int(sim.global_time)


# revision 5
# speedup vs baseline: 1.0324x; 1.0316x over previous
# GCN (2-layer GCNConv + BatchNorm + ReLU + global mean pool) on 8 TRN2 cores.
#
# v4 strategy (dst-partitioned, no ReduceScatter):
#   * Edges partitioned by DST owner.  Every core holds the FULL raw-x row
#     table (host-replicated, pure permutation/padding) in DRAM, split in two
#     25088-row halves so int16 gather indices stay in range.
#   * Core k owns nodes [k*6250,(k+1)*6250); node l -> slot s (s = l, shifted
#     +459 for the upper shard half so each half owns 459 dedicated pad
#     slots); slot s -> partition s%128, group s//128 (56 groups, 28 pairs).
#   * Per-dst-occurrence rounds keep scatter rows unique per instruction.
#     Pipeline per round: Pool gather (f32, src-half substreams joined in one
#     buffer) -> DVE norm-mult (msg * norm_e broadcast over channels, bf16
#     out; norm = dinv[src]*dinv[dst] from index data, 0 on pads) -> Pool
#     scatter-add into an SBUF even/odd-group accumulator pair.
#   * Rounds big enough are split into shard halves A (groups 0..27) and
#     B (28..55); all A pieces run before B so A's aggs assembly + BN-stats
#     matmuls overlap B's edge work.  Mid rounds run unsplit (early).  Deep
#     rounds (small) scatter-add onto a zeroed DRAM strip (per-index cost, no
#     SBUF footprint floor) and fold in via one reload+add before assembly.
#   * aggs = acc + (dinv^2 * x_own + strip);  BN stats via the accumulated
#     A^T[A|1] matmul; cross-core stats via bf16 AllGather + on-chip tree
#     sum; one more bf16 AllGather for the pooled output partials.
#   * Layer 2 never materializes per-node features: out = (q . h1) W2 + b2
#     with q[u,g] built on the host from INDEX data only.
#
# Host-side preprocessing uses only index data (edge_index, batch): degrees,
# edge partitioning/rounds, per-edge norms, the q pooling matrix.  Feature
# data (x) is only permuted/padded on the host, never computed with.

import numpy as np

N_NODES = 50000
N_EDGES = 800000
D = 64
NCORES = 8
NUM_GRAPHS = 64
BN_EPS = 1e-5
SL = N_NODES // NCORES          # 6250 owned nodes per core
NT = 56                         # groups
SLP = 128 * NT                  # 7168 padded slots
HALF_G = NT // 2                # 28 groups per shard half
HALF_S = SLP // 2               # 3584 slots per half
HREAL = SL // 2                 # 3125 real nodes per half
HPAD = HALF_S - HREAL           # 459 pad slots per half
TROWS = 50176                   # padded table rows (2 * 25088)
THALF = TROWS // 2              # 25088 (int16-safe)
STRIP_G = NT + 6                # strip groups (56 real + 6 pad)
STRIP_ROWS = 128 * STRIP_G      # 7552
SBUF_MIN = 1500                 # min real rows for the SBUF scatter path


class Cfg:
    def __init__(self):
        # pieces: execution-ordered dicts
        #  {'kind':'S'|'D', 'n', 'nA', 'nB', 'cols':(c0,c1) pair-col slice,
        #   'stage':0 cross/1 halfA/2 halfB/-1 deep, 'goff','soff','noff'}
        self.pieces = []


LAST_EXEC_TIME_NS = None
_NC_CACHE = {}
_LAST_IN_MAPS = None


def build(cfg):
    import concourse.mybir as mybir
    import concourse.mybir as mb
    import concourse.tile as tile
    from concourse import bacc
    from concourse.masks import make_identity

    f32 = mybir.dt.float32
    bf16 = mybir.dt.bfloat16
    i16 = mybir.dt.int16
    NN = float(N_NODES)
    RG = [list(range(NCORES))]
    pieces = cfg.pieces
    tot_s = sum(p["n"] for p in pieces)
    SMAXC = max(p["n"] for p in pieces)

    nc = bacc.Bacc(
        "TRN2", target_bir_lowering=False, debug=False, num_devices=NCORES
    )

    # --- external inputs ---
    xr = nc.declare_dram_parameter("xr", [TROWS, D], f32, isOutput=False)
    xo = nc.declare_dram_parameter("xo", [128, NT * D], f32, isOutput=False)
    d2o = nc.declare_dram_parameter("d2o", [128, NT], f32, isOutput=False)
    qsl = nc.declare_dram_parameter("qsl", [128, NT * NUM_GRAPHS], bf16,
                                    isOutput=False)
    glo_d = nc.declare_dram_parameter("glo", [128, tot_s // 16], i16,
                                      isOutput=False)
    sct_d = nc.declare_dram_parameter("sct", [128, tot_s // 16], i16,
                                      isOutput=False)
    nrm_d = nc.declare_dram_parameter("nrm", [128, tot_s // 128], f32,
                                      isOutput=False)
    p1_d = nc.declare_dram_parameter("p1", [1, NUM_GRAPHS], f32, isOutput=False)
    w1_d = nc.declare_dram_parameter("w1", [D, D], f32, isOutput=False)
    b1_d = nc.declare_dram_parameter("b1", [D, 1], f32, isOutput=False)
    ga_d = nc.declare_dram_parameter("ga", [D, 1], f32, isOutput=False)
    be_d = nc.declare_dram_parameter("be", [D, 1], f32, isOutput=False)
    w2_d = nc.declare_dram_parameter("w2", [D, D], f32, isOutput=False)
    b2_d = nc.declare_dram_parameter("b2", [1, D], f32, isOutput=False)
    out_d = nc.declare_dram_parameter("out", [NUM_GRAPHS, D], f32,
                                      isOutput=True)

    # --- internal DRAM ---
    strip = nc.dram_tensor("strip", [STRIP_ROWS, 2 * D], bf16)
    sag_in = nc.dram_tensor("sag_in", [D, D + 1], bf16)
    sag_out = nc.dram_tensor("sag_out", [NCORES * D, D + 1], bf16,
                             addr_space="Shared")
    oag_in = nc.dram_tensor("oag_in", [NUM_GRAPHS, D], bf16)
    oag_out = nc.dram_tensor("oag_out", [NCORES * NUM_GRAPHS, D], bf16,
                             addr_space="Shared")

    xr_rows = xr[:, :]
    strip_rows = strip[:, :]

    with tile.TileContext(nc) as tc:
        with (
            tc.tile_pool(name="const", bufs=1) as const,
            tc.tile_pool(name="persist", bufs=1) as persist,
            tc.tile_pool(name="work", bufs=3) as work,
            tc.tile_pool(name="idxp", bufs=3) as idxp,
            tc.tile_pool(name="msgp", bufs=4) as msgp,
            tc.tile_pool(name="msgbp", bufs=4) as msgbp,
            tc.tile_pool(name="spsum", bufs=1, space="PSUM") as spsum,
            tc.tile_pool(name="wpsum", bufs=3, space="PSUM") as wpsum,
        ):
            # --- accumulators + zeroed strip (first: unblock stage-0/deep) ---
            own = persist.tile([128, HALF_G, D], bf16, name="own")    # even g
            peer = persist.tile([128, HALF_G, D], bf16, name="peer")  # odd g
            nc.vector.memset(own[:], 0.0)
            nc.vector.memset(peer[:], 0.0)
            zs = persist.tile([128, 8, 2 * D], bf16, name="zs")
            nc.vector.memset(zs[:], 0.0)
            epsc = const.tile([D, 1], f32)
            nc.vector.memset(epsc[:], BN_EPS)
            strip_v = strip_rows.rearrange("(g p) c -> p g c", p=128)
            for g0 in range(0, STRIP_G, 8):
                gn = min(8, STRIP_G - g0)
                nc.scalar.dma_start(out=strip_v[:, g0 : g0 + gn, :],
                                    in_=zs[:, :gn, :])
            # preload the Sqrt/Relu act tables off the critical path
            warm = const.tile([1, 1], f32)
            nc.scalar.activation(warm[:], epsc[0:1, :],
                                 mb.ActivationFunctionType.Sqrt)
            nc.scalar.activation(warm[:], epsc[0:1, :],
                                 mb.ActivationFunctionType.Relu)

            # --- batched idx/norm loads (chunked, execution order); chunk 0
            # on SP covers the whole stage-0 + deep prefix ---
            glo_s = persist.tile([128, tot_s // 16], i16, name="glo_s")
            sct_s = persist.tile([128, tot_s // 16], i16, name="sct_s")
            nrm_s = persist.tile([128, tot_s // 128], f32, name="nrm_s")
            ICHUNK = 1792
            bounds = [0, 256] + list(range(ICHUNK, tot_s // 16, ICHUNK)) + [
                tot_s // 16]
            for qi in range(len(bounds) - 1):
                c0, c1 = bounds[qi], bounds[qi + 1]
                if c0 >= c1:
                    continue
                eng = nc.sync if qi % 2 == 0 else nc.scalar
                eng.dma_start(out=glo_s[:, c0:c1], in_=glo_d[:, c0:c1])
                eng.dma_start(out=sct_s[:, c0:c1], in_=sct_d[:, c0:c1])
                d0, d1 = c0 // 8, min(c1 // 8, tot_s // 128)
                if qi == len(bounds) - 2:
                    d1 = tot_s // 128
                if d0 < d1:
                    eng.dma_start(out=nrm_s[:, d0:d1], in_=nrm_d[:, d0:d1])

            # --- constants (scalar queue, after the idx chunks) ---
            w1s = const.tile([D, D], f32)
            nc.scalar.dma_start(out=w1s[:], in_=w1_d[:, :])
            w2s = const.tile([D, D], f32)
            nc.scalar.dma_start(out=w2s[:], in_=w2_d[:, :])
            b1c = const.tile([D, 1], f32)
            nc.scalar.dma_start(out=b1c[:], in_=b1_d[:, :])
            gac = const.tile([D, 1], f32)
            nc.scalar.dma_start(out=gac[:], in_=ga_d[:, :])
            bec = const.tile([D, 1], f32)
            nc.scalar.dma_start(out=bec[:], in_=be_d[:, :])
            b2r = const.tile([1, D], f32)
            nc.scalar.dma_start(out=b2r[:], in_=b2_d[:, :])
            p1s = const.tile([1, NUM_GRAPHS], f32)
            nc.scalar.dma_start(out=p1s[:], in_=p1_d[:, :])
            d2s = const.tile([128, NT], f32)
            nc.scalar.dma_start(out=d2s[:], in_=d2o[:, :])
            xos = persist.tile([128, NT, D], f32, name="xos")
            nc.scalar.dma_start(
                out=xos[:], in_=xo[:, :].rearrange("p (g d) -> p g d", d=D)
            )
            qs = persist.tile([128, NT, NUM_GRAPHS], bf16, name="qs")
            nc.scalar.dma_start(
                out=qs[:], in_=qsl[:, :].rearrange("p (g d) -> p g d",
                                                   d=NUM_GRAPHS)
            )

            # --- edge pipeline ---
            def edge_piece(pc):
                n = pc["n"]
                nA = pc["nA"]
                so = pc["soff"]
                no = pc["noff"]
                msg = msgp.tile([128, SMAXC // 128, D], f32, tag="msg",
                                name="msg")
                for half, go, nseg in ((0, 0, nA), (1, nA, n - nA)):
                    if nseg == 0:
                        continue
                    base = half * THALF
                    nc.gpsimd.dma_gather(
                        out_ap=msg[:, go // 128 : (go + nseg) // 128, :],
                        in_ap=xr_rows[base : base + THALF, :],
                        idxs_ap=glo_s[:, so + go // 16 : so + (go + nseg) // 16],
                        num_idxs=nseg, num_idxs_reg=nseg, elem_size=D,
                        single_packet=False, queue_num=0,
                    )
                msgb = msgbp.tile([128, SMAXC // 128, D], bf16, tag="msgb",
                                  name="msgb")
                nc.vector.tensor_tensor(
                    out=msgb[:, : n // 128, :], in0=msg[:, : n // 128, :],
                    in1=nrm_s[:, no : no + n // 128].rearrange(
                        "p (g o) -> p g o", o=1).to_broadcast(
                        [128, n // 128, D]),
                    op=mybir.AluOpType.mult,
                )
                if pc["kind"] == "S":
                    c0, c1 = pc["cols"]
                    nc.gpsimd.dma_scatter_add(
                        own[:, c0:c1, :], msgb[:, : n // 128, :],
                        sct_s[:, so : so + n // 16], n, n, D,
                        sbuf_tokens_per_rank=128, parity_reg=0,
                        out_ap_other=peer[:, c0:c1, :],
                        single_packet=False, queue_num=0,
                    )
                else:
                    nc.gpsimd.dma_scatter_add(
                        strip_rows[:, 0:D], msgb[:, : n // 128, :],
                        sct_s[:, so : so + n // 16], n, n, D,
                        elem_step=2 * D,
                        single_packet=False, queue_num=0,
                    )

            def deep_bundle(pcs):
                """One gather stream + one norm-mult feeding several per-round
                strip scatters (avoids tiny chained pieces)."""
                n = sum(p["n"] for p in pcs)
                so0 = pcs[0]["soff"]
                no0 = pcs[0]["noff"]
                msg = msgp.tile([128, SMAXC // 128, D], f32, tag="msg",
                                name="msg")
                off = 0
                for pc in pcs:
                    for half, go, nseg in ((0, 0, pc["nA"]),
                                           (1, pc["nA"], pc["n"] - pc["nA"])):
                        if nseg == 0:
                            continue
                        base = half * THALF
                        o = off + go
                        nc.gpsimd.dma_gather(
                            out_ap=msg[:, o // 128 : (o + nseg) // 128, :],
                            in_ap=xr_rows[base : base + THALF, :],
                            idxs_ap=glo_s[:, so0 + o // 16 :
                                          so0 + (o + nseg) // 16],
                            num_idxs=nseg, num_idxs_reg=nseg, elem_size=D,
                            single_packet=False, queue_num=0,
                        )
                    off += pc["n"]
                msgb = msgbp.tile([128, SMAXC // 128, D], bf16, tag="msgb",
                                  name="msgb")
                nc.vector.tensor_tensor(
                    out=msgb[:, : n // 128, :], in0=msg[:, : n // 128, :],
                    in1=nrm_s[:, no0 : no0 + n // 128].rearrange(
                        "p (g o) -> p g o", o=1).to_broadcast(
                        [128, n // 128, D]),
                    op=mybir.AluOpType.mult,
                )
                off = 0
                for pc in pcs:
                    pn = pc["n"]
                    nc.gpsimd.dma_scatter_add(
                        strip_rows[:, 0:D],
                        msgb[:, off // 128 : (off + pn) // 128, :],
                        sct_s[:, so0 + off // 16 : so0 + (off + pn) // 16],
                        pn, pn, D, elem_step=2 * D,
                        single_packet=False, queue_num=0,
                    )
                    off += pn

            aggs = persist.tile([128, NT, D + 1], bf16, name="aggs")
            stats_ps = spsum.tile([D, D + 1], f32, name="stats_ps")
            tsl = persist.tile([128, NT, D], f32, name="tsl")
            agv = aggs[:, :, :D].rearrange("p (a two) d -> p a two d", two=2)
            tsv = tsl[:].rearrange("p (a two) d -> p a two d", two=2)

            def assemble(h):
                # chunked adds + stats matmuls so PE overlaps the DVE adds
                q = HALF_G // 2  # 14 pair-cols per half
                for ci in range(0, q, 4):
                    a0 = h * q + ci
                    a1 = min(a0 + 4, (h + 1) * q)
                    nc.vector.tensor_tensor(
                        out=agv[:, a0:a1, 0, :], in0=own[:, a0:a1, :],
                        in1=tsv[:, a0:a1, 0, :], op=mybir.AluOpType.add,
                    )
                    nc.vector.tensor_tensor(
                        out=agv[:, a0:a1, 1, :], in0=peer[:, a0:a1, :],
                        in1=tsv[:, a0:a1, 1, :], op=mybir.AluOpType.add,
                    )
                    for g in range(2 * a0, 2 * a1):
                        nc.tensor.matmul(
                            out=stats_ps[:], lhsT=aggs[:, g, :D],
                            rhs=aggs[:, g, :],
                            start=(g == 0), stop=(g == NT - 1),
                        )

            # emit pieces in plan (execution) order; deep pieces are
            # contiguous and bundled; strip fold lands after the last deep
            # piece; assembly A runs inside half B
            def emit_mid_consts():
                ident = const.tile([128, 128], f32)
                make_identity(nc, ident[:])
                identb = const.tile([128, 128], bf16)
                nc.vector.tensor_copy(out=identb[:], in_=ident[:])
                w1b = const.tile([D, D], bf16)
                nc.vector.tensor_copy(out=w1b[:], in_=w1s[:])
                ones64 = const.tile([D, 1], f32)
                nc.vector.memset(ones64[:], 1.0)
                b1sq = persist.tile([D, 1], f32, name="b1sq")
                nc.vector.tensor_tensor(out=b1sq[:], in0=b1c[:], in1=b1c[:],
                                        op=mybir.AluOpType.mult)
                nc.vector.memset(aggs[:, :, D : D + 1], 1.0)
                b2_ps = wpsum.tile([NUM_GRAPHS, D], f32, tag="ps_b",
                                   name="b2_ps")
                nc.tensor.matmul(out=b2_ps[:], lhsT=p1s[:], rhs=b2r[:],
                                 start=True, stop=True)
                b2m = persist.tile([NUM_GRAPHS, D], f32, name="b2m")
                nc.vector.tensor_copy(out=b2m[:], in_=b2_ps[:])
                return identb, w1b, ones64, b1sq, b2m

            def emit_fold():
                # t = dinv^2*x_own + deep strip (all D pieces done); runs on
                # DVE while Pool continues the remaining rounds
                nc.vector.tensor_tensor(
                    out=tsl[:], in0=xos[:],
                    in1=d2s[:, :].rearrange(
                        "p (g o) -> p g o", o=1).to_broadcast([128, NT, D]),
                    op=mybir.AluOpType.mult,
                )
                stb = persist.tile([128, NT, D], bf16, name="stb")
                nc.scalar.dma_start(
                    out=stb[:],
                    in_=strip_rows.rearrange("(g p) c -> p g c", p=128)[
                        :, 0:NT, 0:D],
                )
                nc.vector.tensor_tensor(out=tsl[:], in0=tsl[:], in1=stb[:],
                                        op=mybir.AluOpType.add)

            last_d = max((i for i, pc in enumerate(pieces)
                          if pc["stage"] == -1), default=-1)
            first_s2 = next((i for i, pc in enumerate(pieces)
                             if pc["stage"] == 2), len(pieces))
            bundle, bn_tot = [], 0
            folded = False
            consts_done = False
            asm0_done = False
            for i, pc in enumerate(pieces):
                if pc["stage"] == -1:
                    if bn_tot + pc["n"] > SMAXC and bundle:
                        deep_bundle(bundle)
                        bundle, bn_tot = [], 0
                    bundle.append(pc)
                    bn_tot += pc["n"]
                else:
                    if bundle:
                        deep_bundle(bundle)
                        bundle, bn_tot = [], 0
                    edge_piece(pc)
                if i >= last_d and not folded:
                    if bundle:
                        deep_bundle(bundle)
                        bundle, bn_tot = [], 0
                    emit_fold()
                    folded = True
                if i >= 1 and not consts_done:
                    identb, w1b, ones64, b1sq, b2m = emit_mid_consts()
                    consts_done = True
                if i >= first_s2 and not asm0_done:
                    assemble(0)  # overlaps half-B edge work
                    asm0_done = True
            if bundle:
                deep_bundle(bundle)
            if not folded:
                emit_fold()
            if not consts_done:
                identb, w1b, ones64, b1sq, b2m = emit_mid_consts()
            if not asm0_done:
                assemble(0)
            assemble(1)
            stats_sb = persist.tile([D, D + 1], bf16, name="stats_sb")
            nc.scalar.activation(stats_sb[:], stats_ps[:],
                                 mb.ActivationFunctionType.Copy)
            nc.sync.dma_start(out=sag_in[:, :], in_=stats_sb[:])
            nc.gpsimd.collective_compute(
                "AllGather", mybir.AluOpType.bypass, replica_groups=RG,
                ins=[sag_in[:, :]], outs=[sag_out[:, :]],
            )

            # --- transposed h (pre-BN) while the AllGather is in flight ---
            hT_big = persist.tile([D, NT * 128], bf16, name="hT_big")
            for b0 in range(0, NT, 4):
                bn = min(4, NT - b0)
                tp_ps = wpsum.tile([D, 512], bf16, tag="ps_a", name="tp_ps")
                for j in range(bn):
                    b = b0 + j
                    nc.tensor.transpose(
                        out=tp_ps[:, j * 128 : (j + 1) * 128],
                        in_=aggs[:, b, :D], identity=identb[:],
                    )
                aggsT = work.tile([D, 512], bf16, tag="aggsT", name="aggsT",
                                  bufs=2)
                nc.vector.tensor_copy(out=aggsT[:, : bn * 128],
                                      in_=tp_ps[:, : bn * 128])
                hT_ps = wpsum.tile([D, 512], f32, tag="ps_b", name="hT_ps")
                nc.tensor.matmul(
                    out=hT_ps[:, : bn * 128], lhsT=w1b[:],
                    rhs=aggsT[:, : bn * 128], start=True, stop=True,
                )
                nc.scalar.activation(
                    hT_big[:, b0 * 128 : (b0 + bn) * 128],
                    hT_ps[:, : bn * 128], mb.ActivationFunctionType.Copy,
                )

            # --- stats tree-sum + BN scalar algebra ---
            st8 = persist.tile([D, NCORES, D + 1], bf16, name="st8")
            nc.sync.dma_start(
                out=st8[:], in_=sag_out[:, :].rearrange("(r p) c -> p r c",
                                                        p=D)
            )
            st4 = persist.tile([D, 4, D + 1], f32, name="st4")
            nc.vector.tensor_tensor(
                out=st4[:], in0=st8[:, 0:4, :], in1=st8[:, 4:8, :],
                op=mybir.AluOpType.add,
            )
            nc.vector.tensor_tensor(
                out=st4[:, 0:2, :], in0=st4[:, 0:2, :], in1=st4[:, 2:4, :],
                op=mybir.AluOpType.add,
            )
            st = persist.tile([D, D + 1], f32, name="st")
            nc.vector.tensor_tensor(
                out=st[:], in0=st4[:, 0, :], in1=st4[:, 1, :],
                op=mybir.AluOpType.add,
            )

            q_ps = wpsum.tile([D, 1], f32, tag="ps_a", name="q_ps")
            nc.tensor.matmul(out=q_ps[:], lhsT=w1s[:], rhs=st[:, D : D + 1],
                             start=True, stop=True)
            mu = persist.tile([D, 1], f32, name="mu")
            nc.vector.tensor_scalar(
                out=mu[:], in0=q_ps[:], scalar1=1.0 / NN, scalar2=b1c[:],
                op0=mybir.AluOpType.mult, op1=mybir.AluOpType.add,
            )
            t1_ps = wpsum.tile([D, D], f32, tag="ps_b", name="t1_ps")
            nc.tensor.matmul(out=t1_ps[:], lhsT=st[:, :D], rhs=w1s[:],
                             start=True, stop=True)
            m_sb = work.tile([D, D], f32, tag="m_sb", name="m_sb")
            nc.vector.tensor_tensor(out=m_sb[:], in0=w1s[:], in1=t1_ps[:],
                                    op=mybir.AluOpType.mult)
            d_ps = wpsum.tile([D, 1], f32, tag="ps_b", name="d_ps")
            nc.tensor.matmul(out=d_ps[:], lhsT=m_sb[:], rhs=ones64[:],
                             start=True, stop=True)

            var = persist.tile([D, 1], f32, name="var")
            t2 = work.tile([D, 1], f32, tag="t2", name="t2")
            nc.vector.tensor_scalar(
                out=t2[:], in0=q_ps[:], scalar1=2.0 / NN, scalar2=b1c[:],
                op0=mybir.AluOpType.mult, op1=mybir.AluOpType.mult,
            )
            nc.vector.tensor_scalar(
                out=var[:], in0=d_ps[:], scalar1=1.0 / NN, scalar2=t2[:],
                op0=mybir.AluOpType.mult, op1=mybir.AluOpType.add,
            )
            nc.vector.tensor_tensor(out=var[:], in0=var[:], in1=b1sq[:],
                                    op=mybir.AluOpType.add)
            t4 = work.tile([D, 1], f32, tag="t4", name="t4")
            nc.vector.tensor_tensor(out=t4[:], in0=mu[:], in1=mu[:],
                                    op=mybir.AluOpType.mult)
            nc.vector.tensor_tensor(out=var[:], in0=var[:], in1=t4[:],
                                    op=mybir.AluOpType.subtract)

            sd = work.tile([D, 1], f32, tag="sd", name="sd")
            nc.scalar.activation(sd[:], var[:], mb.ActivationFunctionType.Sqrt,
                                 bias=epsc[:])
            rstd = work.tile([D, 1], f32, tag="rstd", name="rstd")
            nc.vector.reciprocal(out=rstd[:], in_=sd[:])
            a_sb = persist.tile([D, 1], f32, name="a_sb")
            nc.vector.tensor_tensor(out=a_sb[:], in0=gac[:], in1=rstd[:],
                                    op=mybir.AluOpType.mult)
            c_sb = persist.tile([D, 1], f32, name="c_sb")
            t5 = work.tile([D, 1], f32, tag="t5", name="t5")
            nc.vector.tensor_tensor(out=t5[:], in0=mu[:], in1=a_sb[:],
                                    op=mybir.AluOpType.mult)
            nc.vector.tensor_tensor(out=c_sb[:], in0=bec[:], in1=t5[:],
                                    op=mybir.AluOpType.subtract)
            # hT excludes the b1 bias; fold it into the BN offset
            t6 = work.tile([D, 1], f32, tag="t6", name="t6")
            nc.vector.tensor_tensor(out=t6[:], in0=a_sb[:], in1=b1c[:],
                                    op=mybir.AluOpType.mult)
            nc.vector.tensor_tensor(out=c_sb[:], in0=c_sb[:], in1=t6[:],
                                    op=mybir.AluOpType.add)

            # --- BN+ReLU, transpose back, pool matmul ---
            h1 = persist.tile([128, NT, D], bf16, name="h1")
            poolT_ps = spsum.tile([D, NUM_GRAPHS], f32, name="poolT_ps")
            for b0 in range(0, NT, 4):
                bn = min(4, NT - b0)
                h1T = work.tile([D, 512], bf16, tag="h1T", name="h1T", bufs=2)
                nc.scalar.activation(
                    h1T[:, : bn * 128],
                    hT_big[:, b0 * 128 : (b0 + bn) * 128],
                    mb.ActivationFunctionType.Relu, bias=c_sb[:], scale=a_sb[:],
                )
                for j in range(bn):
                    b = b0 + j
                    nm_ps = wpsum.tile([128, D], bf16, tag="ps_a", name="nm_ps")
                    nc.tensor.transpose(
                        out=nm_ps[:], in_=h1T[:, j * 128 : (j + 1) * 128],
                        identity=identb[:D, :D],
                    )
                    nc.vector.tensor_copy(out=h1[:, b, :], in_=nm_ps[:])
                    nc.tensor.matmul(
                        out=poolT_ps[:], lhsT=h1[:, b, :], rhs=qs[:, b, :],
                        start=(b == 0), stop=(b == NT - 1),
                    )

            # --- out partial, AllGather (bf16), tree sum, +b2, store ---
            poolT_sb = persist.tile([D, NUM_GRAPHS], f32, name="poolT_sb")
            nc.vector.tensor_copy(out=poolT_sb[:], in_=poolT_ps[:])
            out_ps = wpsum.tile([NUM_GRAPHS, D], f32, tag="ps_b", name="out_ps")
            nc.tensor.matmul(out=out_ps[:], lhsT=poolT_sb[:], rhs=w2s[:],
                             start=True, stop=True)
            out_sb = persist.tile([NUM_GRAPHS, D], bf16, name="out_sb")
            nc.vector.tensor_tensor(out=out_sb[:], in0=out_ps[:], in1=b2m[:],
                                    op=mybir.AluOpType.add)
            nc.sync.dma_start(out=oag_in[:, :], in_=out_sb[:])
            nc.gpsimd.collective_compute(
                "AllGather", mybir.AluOpType.bypass, replica_groups=RG,
                ins=[oag_in[:, :]], outs=[oag_out[:, :]],
            )
            o8 = persist.tile([NUM_GRAPHS, NCORES, D], bf16, name="o8")
            nc.sync.dma_start(
                out=o8[:],
                in_=oag_out[:, :].rearrange("(r p) c -> p r c", p=NUM_GRAPHS),
            )
            o4 = persist.tile([NUM_GRAPHS, 4, D], f32, name="o4")
            nc.vector.tensor_tensor(
                out=o4[:], in0=o8[:, 0:4, :], in1=o8[:, 4:8, :],
                op=mybir.AluOpType.add,
            )
            nc.vector.tensor_tensor(
                out=o4[:, 0:2, :], in0=o4[:, 0:2, :], in1=o4[:, 2:4, :],
                op=mybir.AluOpType.add,
            )
            outf = persist.tile([NUM_GRAPHS, D], f32, name="outf")
            nc.vector.tensor_tensor(
                out=outf[:], in0=o4[:, 0, :], in1=o4[:, 1, :],
                op=mybir.AluOpType.add,
            )
            nc.sync.dma_start(out=out_d[:, :], in_=outf[:])

    nc.compile()
    return nc


def _wrap16(v, n):
    """idx j at [j%16, j//16], replicated to 128 partitions (8 Q7 cores)."""
    assert v.shape[0] == n and n % 16 == 0
    t = v.astype(np.int16).reshape(n // 16, 16).T
    return np.tile(t, (8, 1))


def _wrap128(v, n):
    """value j at [j%128, j//128] (norm layout for the gather stream)."""
    assert v.shape[0] == n and n % 128 == 0
    return np.ascontiguousarray(v.astype(np.float32).reshape(n // 128, 128).T)


def _up128(v):
    return ((v + 127) // 128) * 128 if v else 0


def _slot_of(l):
    """node local id -> slot (upper half shifted past half-A pad zone)."""
    return np.where(l < HREAL, l, l + HPAD)


def prepare_inputs(cfg, x, edge_index, batch, W1, b1, gamma, beta, W2, b2):
    """Host-side index preprocessing + per-core input maps. Fills cfg.pieces."""
    x = np.ascontiguousarray(np.asarray(x, dtype=np.float32))
    src = np.asarray(edge_index[0], dtype=np.int64)
    dst = np.asarray(edge_index[1], dtype=np.int64)
    batch = np.asarray(batch, dtype=np.int64)
    W1 = np.asarray(W1, dtype=np.float32)
    b1 = np.asarray(b1, dtype=np.float32)
    gamma = np.asarray(gamma, dtype=np.float32)
    beta = np.asarray(beta, dtype=np.float32)
    W2 = np.asarray(W2, dtype=np.float32)
    b2 = np.asarray(b2, dtype=np.float32)
    n = N_NODES

    deg = np.bincount(dst, minlength=n).astype(np.float64) + 1.0  # + self-loop
    dinv = 1.0 / np.sqrt(deg)

    cnt = np.bincount(batch, minlength=NUM_GRAPHS).astype(np.float64)
    w_graph = 1.0 / np.maximum(cnt, 1.0)

    # q pooling matrix for layer 2 (index data only)
    wg = w_graph[batch]
    q = np.bincount(
        src * NUM_GRAPHS + batch[dst],
        weights=dinv[src] * dinv[dst] * wg[dst],
        minlength=n * NUM_GRAPHS,
    )
    q += np.bincount(
        np.arange(n) * NUM_GRAPHS + batch,
        weights=dinv * dinv * wg,
        minlength=n * NUM_GRAPHS,
    )
    q = q.reshape(n, NUM_GRAPHS).astype(np.float32)
    # scaled by 1/NCORES: every core folds b2m/8 into its partial before the
    # AllGather, so the summed partials already contain the full b2 bias
    p1 = ((cnt > 0).astype(np.float32) / NCORES).reshape(1, NUM_GRAPHS)
    norm_all = (dinv[src] * dinv[dst]).astype(np.float32)
    dinv = dinv.astype(np.float32)

    xr = np.zeros((TROWS, D), dtype=np.float32)
    xr[:n] = x

    import ml_dtypes

    # per-core rounds: (slots, srcs, norms, nA=src<THALF count), A-first order
    per_core = []
    for k in range(NCORES):
        sel = (dst >= k * SL) & (dst < (k + 1) * SL)
        sl_d = _slot_of(dst[sel] - k * SL)
        sr = src[sel]
        nm = norm_all[sel]
        order = np.argsort(sl_d, kind="stable")
        sl_d, sr, nm = sl_d[order], sr[order], nm[order]
        chg = np.r_[True, sl_d[1:] != sl_d[:-1]] if len(sl_d) else np.zeros(
            0, bool)
        starts = np.flatnonzero(chg)
        gg = np.cumsum(chg) - 1
        occ = (np.arange(len(sl_d)) - starts[gg]) if len(sl_d) else np.zeros(
            0, np.int64)
        nr = int(occ.max()) + 1 if len(occ) else 0
        rounds = []
        for r in range(nr):
            m = occ == r
            rounds.append((sl_d[m], sr[m], nm[m]))
        per_core.append(rounds)
    nr_max = max(len(rc) for rc in per_core)

    def parts(rc, r, half_sel):
        if r >= len(rc):
            e = np.zeros(0, np.int64)
            return e, e, np.zeros(0, np.float32), 0
        sl_d, sr, nm = rc[r]
        if half_sel is not None:
            m = (sl_d // HALF_S) == half_sel
            sl_d, sr, nm = sl_d[m], sr[m], nm[m]
        ha = sr < THALF
        return (np.r_[sl_d[ha], sl_d[~ha]], np.r_[sr[ha], sr[~ha]],
                np.r_[nm[ha], nm[~ha]].astype(np.float32), int(ha.sum()))

    # piece plan (identical instruction shapes on every core), in EXECUTION
    # order so the chunked idx loads stream in the order pieces consume them.
    # Small cross-half (stage 0) and deep (stage -1) pieces are interleaved
    # among the first half-A rounds: their DVE norm-mults hide under the big
    # rounds' Pool gather/scatter time instead of serializing at the start.
    p_mid, p_deep, p_a, p_b = [], [], [], []
    for r in range(nr_max):
        nmax = max(len(rc[r][0]) if r < len(rc) else 0 for rc in per_core)
        if nmax >= 2 * SBUF_MIN:
            p_a.append(("S", r, 0, 1))
            p_b.append(("S", r, 1, 2))
        elif nmax >= SBUF_MIN:
            p_mid.append(("S", r, None, 0))
        else:
            p_deep.append(("D", r, None, -1))
    plans = []
    ins_a = list(p_a)
    plans.append(ins_a.pop(0)) if ins_a else None
    for m in p_mid:
        plans.append(m)
        if ins_a:
            plans.append(ins_a.pop(0))
    plans.extend(p_deep)  # contiguous: device bundles them
    plans.extend(ins_a)
    plans.extend(p_b)

    cfg.pieces = []
    core_arrs = [([], [], []) for _ in range(NCORES)]
    off = 0
    for kind, r, half_sel, stage in plans:
        nA = _up128(max(parts(rc, r, half_sel)[3] for rc in per_core))
        nB = _up128(max(len(parts(rc, r, half_sel)[0])
                        - parts(rc, r, half_sel)[3] for rc in per_core))
        ntot = nA + nB
        if ntot == 0:
            continue
        if kind == "S":
            cols = (0, HALF_G // 2) if half_sel == 0 else (
                (HALF_G // 2, HALF_G) if half_sel == 1 else (0, HALF_G))
        else:
            cols = None
        cfg.pieces.append(
            {"kind": kind, "n": ntot, "nA": nA, "cols": cols, "stage": stage,
             "goff": off // 16, "soff": off // 16, "noff": off // 128,
             "rnd": r}
        )
        for k in range(NCORES):
            sl_d, sr, nm, nAr = parts(per_core[k], r, half_sel)
            nBr = len(sl_d) - nAr
            ga = np.zeros(ntot, dtype=np.int64)
            na = np.zeros(ntot, dtype=np.float32)
            sa = np.zeros(ntot, dtype=np.int64)
            ga[:nAr] = sr[:nAr]
            na[:nAr] = nm[:nAr]
            ga[nA : nA + nBr] = sr[nAr:] - THALF
            na[nA : nA + nBr] = nm[nAr:]
            pad_pos = np.r_[np.arange(nAr, nA), np.arange(nA + nBr, ntot)]
            if kind == "S":
                c0 = cols[0]

                def enc(s):
                    p = s % 128
                    g = s // 128
                    return (((g >> 1) - c0) * 2 + (g & 1)) * 128 + p

                sa[:nAr] = enc(sl_d[:nAr])
                sa[nA : nA + nBr] = enc(sl_d[nAr:])
                if len(pad_pos):
                    # dedicated pad slots inside this piece's col range
                    pad_base = HREAL if cols[0] == 0 else HALF_S + HREAL
                    avail = HPAD
                    assert len(pad_pos) <= avail, (len(pad_pos), avail)
                    sa[pad_pos] = enc(pad_base + np.arange(len(pad_pos)))
            else:
                sa[:nAr] = sl_d[:nAr]
                sa[nA : nA + nBr] = sl_d[nAr:]
                assert len(pad_pos) <= STRIP_ROWS - SLP
                sa[pad_pos] = SLP + np.arange(len(pad_pos))
            core_arrs[k][0].append(_wrap16(ga, ntot))
            core_arrs[k][1].append(_wrap16(sa, ntot))
            core_arrs[k][2].append(_wrap128(na, ntot))
        off += ntot

    # per-core node-major own arrays (slot layout)
    in_maps = []
    ll = np.arange(SL)
    ss = _slot_of(ll)
    P_arr = ss % 128
    G_arr = ss // 128
    for k in range(NCORES):
        lo = k * SL
        xo = np.zeros((128, NT, D), dtype=np.float32)
        xo[P_arr, G_arr] = x[lo : lo + SL]
        qslc = np.zeros((128, NT, NUM_GRAPHS), dtype=np.float32)
        qslc[P_arr, G_arr] = q[lo : lo + SL]
        d2 = np.zeros((128, NT), dtype=np.float32)
        d2[P_arr, G_arr] = dinv[lo : lo + SL] ** 2
        im = {
            "xr": xr,
            "xo": np.ascontiguousarray(xo.reshape(128, NT * D)),
            "d2o": d2,
            "qsl": np.ascontiguousarray(
                qslc.reshape(128, NT * NUM_GRAPHS).astype(ml_dtypes.bfloat16)),
            "glo": np.concatenate(core_arrs[k][0], axis=1),
            "sct": np.concatenate(core_arrs[k][1], axis=1),
            "nrm": np.concatenate(core_arrs[k][2], axis=1),
            "p1": p1,
            "w1": W1,
            "b1": b1.reshape(D, 1),
            "ga": gamma.reshape(D, 1),
            "be": beta.reshape(D, 1),
            "w2": W2,
            "b2": b2.reshape(1, D),
        }
        in_maps.append(im)
    return in_maps


def kernel(x, edge_index, batch, W1, b1, gamma, beta, W2, b2):
    global LAST_EXEC_TIME_NS, _LAST_IN_MAPS
    from concourse.bass_utils import run_bass_kernel_spmd

    cfg = Cfg()
    in_maps = prepare_inputs(cfg, x, edge_index, batch, W1, b1, gamma, beta,
                             W2, b2)

    key = tuple((p["kind"], p["n"], p["nA"], p["cols"], p["stage"])
                for p in cfg.pieces)
    if key not in _NC_CACHE:
        _NC_CACHE[key] = build(cfg)
    nc = _NC_CACHE[key]
    _LAST_IN_MAPS = in_maps

    res = run_bass_kernel_spmd(nc, in_maps, list(range(NCORES)), trace=False)
    LAST_EXEC_TIME_NS = res.exec_time_ns
    return np.asarray(res.results[0]["out"], dtype=np.float32)


def modeled_time_ns(**kw):
    """Cost-model execution time (MultiCoreSim, mocked collectives)."""
    if not _NC_CACHE:
        return None
    nc = next(iter(_NC_CACHE.values()))
    ins = _LAST_IN_MAPS
    if ins is None:
        return None
    from concourse.bass_interp import MultiCoreSim

    sim = MultiCoreSim(nc, 2, debug_mock_collectives_without_correctness=True)
    for i, core in sim.cores.items():
        for name, val in ins[i].items():
            core.tensor(name)[:] = val
    sim.simulate()
    return int(sim.global_time)
